# revision 1
# baseline (speedup 1.0000x reference)
"""Trainium2 Bass kernel for nn_BertSelfAttention_79577154060613.

Block-sparse BERT self-attention (block-diagonal over 10 candidate blocks of
64 tokens + dense global columns for 128 term tokens), data-parallel over
batch across 8 NeuronCores (2 batches per core).

Key algorithmic trick: the reference multiplies scores by the mask (masked
entries become exactly 0, not -inf), so softmax gives each masked key weight
exp(0)=1. For a query in block c:
    ctx = (sum_{k in block c | terms} e^{s_k} v_k + sum_{c' != c} Vsum_c') / Z
    Z   = sum_{k in block c | terms} e^{s_k} + 9*64
where Vsum_c' are per-head, per-block sums of candidate value rows. This
turns 768-wide attention into 192-wide attention plus one small K=10 matmul
(lhsT = 1 - one_hot(c)) per query tile.

All tensor-engine inputs are bf16 (fp32 matmuls stream at 1/4 rate on TRN2);
accumulation stays fp32 in PSUM and the softmax divide runs in fp32, so the
end-to-end error stays at the bf16-rounding level (~2e-3 relative).

Layouts (no on-chip transposes anywhere):
  - X^T [h, t]: host pre-transposes and pre-casts hidden_states.
  - Q^T, K^T [o, t] = matmul(lhsT=W^T tile, rhs=X^T); head h lives at
    partitions (h%2)*64 of tile h//2. Biases are added by the PSUM->SBUF
    copy (per-partition tensor_scalar add).
  - V [t, o] = matmul(lhsT=X^T tile, rhs=W^T), stored per head with a ones
    column ([t, 12*(64+1)] bf16) so every PV matmul also accumulates the
    softmax denominator into a 65th PSUM column. V's bias (free-dim) is
    added via a materialized [128, H] bias tile on the copy.
  - scores^T [k, q] = matmul(lhsT=K^T head, rhs=Q^T head); exp on ACT
    (scale=1/8) casting to bf16; the exp'ed scores are the *stationary*
    operand of PV, giving ctx in natural [q, dh] layout, so the divided
    output DMAs straight out.

PSUM discipline: start=True lazily zeroes the whole 2KB bank for the written
partitions, so each bank hosts exactly one accumulation group per partition
half, opened by the correction matmuls (which cover all 4 heads' columns).
"""

import numpy as np
import ml_dtypes

import concourse.bass as bass
import concourse.mybir as mybir
import concourse.tile as tile
from concourse import bacc
from concourse.bass_utils import run_bass_kernel_spmd

# Problem dims (hardcoded per contract)
B, CDD, L, T, H, NH = 16, 10, 64, 128, 768, 12
DH = H // NH  # 64
S = CDD * L + T  # 768
NQ = CDD * L  # 640
P = 128
NCORES = 8
BL = B // NCORES  # 2 batches per core
KT = H // P  # 6 contraction tiles
FP32 = mybir.dt.float32
BF16 = mybir.dt.bfloat16
AF = mybir.ActivationFunctionType
ALU = mybir.AluOpType
HGS = 4  # heads per attention group
NHG = NH // HGS  # 3 groups
VW = DH + 1  # value width per head incl. ones column (65)


def _build_program():
    nc = bacc.Bacc(
        "TRN2", target_bir_lowering=False, debug=False, num_devices=NCORES
    )
    x = nc.dram_tensor("x", [BL, H, S], BF16, kind="ExternalInput").ap()
    wqt = nc.dram_tensor("wqt", [H, H], BF16, kind="ExternalInput").ap()
    wkt = nc.dram_tensor("wkt", [H, H], BF16, kind="ExternalInput").ap()
    wvt = nc.dram_tensor("wvt", [H, H], BF16, kind="ExternalInput").ap()
    bq = nc.dram_tensor("bq", [H], FP32, kind="ExternalInput").ap()
    bk = nc.dram_tensor("bk", [H], FP32, kind="ExternalInput").ap()
    bv16 = nc.dram_tensor("bv16", [H], BF16, kind="ExternalInput").ap()
    out = nc.dram_tensor("out", [BL, S, H], FP32, kind="ExternalOutput").ap()

    with tile.TileContext(nc) as tc:
        _emit(tc, nc, x, wqt, wkt, wvt, bq, bk, bv16, out)
    nc.compile()
    return nc


def _emit(tc, nc, x, wqt, wkt, wvt, bq, bk, bv16, out):
    from contextlib import ExitStack

    ctx = ExitStack()
    with ctx:
        cpool = ctx.enter_context(tc.tile_pool(name="consts", bufs=1))
        wpool = ctx.enter_context(tc.tile_pool(name="weights", bufs=1))
        xtp = ctx.enter_context(tc.tile_pool(name="xt", bufs=2))
        qkv = ctx.enter_context(tc.tile_pool(name="qkv", bufs=2))
        sep = ctx.enter_context(tc.tile_pool(name="se", bufs=3))
        osp = ctx.enter_context(tc.tile_pool(name="osb", bufs=1))
        smp = ctx.enter_context(tc.tile_pool(name="small", bufs=2))
        psp = ctx.enter_context(tc.tile_pool(name="psum", bufs=1, space="PSUM"))

        # ---- constants ----
        onesrow = cpool.tile([1, P], BF16)  # 1.0 row (rank-1 lhsT)
        nc.gpsimd.memset(onesrow[:], 1.0)
        zrow = cpool.tile([1, 1], BF16)  # 0.0 (group-closer rank-1 rhs)
        nc.gpsimd.memset(zrow[:], 0.0)
        # notselC[p, c*64+j] = 0 if p == c else 1  (p in 0..9)
        notselC = cpool.tile([CDD, NQ], BF16)
        nc.gpsimd.memset(notselC[:], 1.0)
        nc.gpsimd.affine_select(
            out=notselC.rearrange("p (c j) -> p c j", j=L),
            in_=notselC.rearrange("p (c j) -> p c j", j=L),
            compare_op=ALU.not_equal,
            fill=0.0,
            base=0,
            pattern=[[-1, CDD], [0, L]],
            channel_multiplier=1,
        )
        # block-membership indicator for Vsums: G[p, j] = 1 iff j-10 == p//64
        G = cpool.tile([P, 20], BF16)
        nc.gpsimd.memset(G[:], 0.0)
        nc.gpsimd.memset(G[0:64, 10:11], 1.0)
        nc.gpsimd.memset(G[64:128, 11:12], 1.0)

        # ---- weights & biases (shared by both batches) ----
        # xt(b=0) + wq chunks are interleaved so the first projection's
        # K-accumulation can start as soon as chunk 0 lands; wk/wv follow.
        w_sb = {}
        w_aps = {"q": wqt, "k": wkt, "v": wvt}
        for name in ("q", "k", "v"):
            w_sb[name] = wpool.tile(
                [P, KT, H], BF16, tag=f"w{name}", name=f"w{name}"
            )
        bvb = cpool.tile([P, H], FP32)  # built right before V projection
        b_col = {}
        bv_row = cpool.tile([1, H], BF16)
        xt0 = []
        for kt in range(KT):
            t = xtp.tile([P, S], BF16, tag=f"xt{kt}", name=f"xt{kt}")
            nc.sync.dma_start(out=t[:], in_=x[0][kt * P : (kt + 1) * P, :])
            nc.sync.dma_start(
                out=w_sb["q"][:, kt, :],
                in_=w_aps["q"].rearrange("(kt p) o -> p kt o", p=P)[:, kt, :],
            )
            xt0.append(t)
            if kt == 0:
                # tiny bias DMAs (needed by the first projection copies)
                for name, bap in (("q", bq), ("k", bk)):
                    bc = cpool.tile([P, KT], FP32, tag=f"bc{name}", name=f"bcol{name}")
                    nc.sync.dma_start(
                        out=bc[:], in_=bap.rearrange("(t p) -> p t", p=P)
                    )
                    b_col[name] = bc
                nc.sync.dma_start(out=bv_row[:], in_=bv16[None, :])
        for name in ("k", "v"):
            wr = w_aps[name].rearrange("(kt p) o -> p kt o", p=P)
            for kt in range(KT):
                nc.sync.dma_start(out=w_sb[name][:, kt, :], in_=wr[:, kt, :])

        xt_next = xt0
        for b in range(BL):
            # ---- X^T (host pre-transposed, bf16; prefetched) ----
            xt = xt_next

            # ---- projections ----
            qt_sb = [qkv.tile([P, NQ], BF16, tag=f"qt{m}", name=f"qt{m}") for m in range(KT)]
            kt_sb = [qkv.tile([P, S], BF16, tag=f"kt{m}", name=f"kt{m}") for m in range(KT)]
            vext = [qkv.tile([P, NH * VW], BF16, tag=f"v{m}", name=f"v{m}") for m in range(KT)]
            vterm = qkv.tile([P, H], FP32, tag="vterm", name="vterm")

            # Q^T, K^T: out[o-tile, t-chunk]; bias added on the copy
            for name, dst, nlen_total in (("q", qt_sb, NQ), ("k", kt_sb, S)):
                for mt in range(KT):
                    ms = slice(mt * P, (mt + 1) * P)
                    n0 = 0
                    while n0 < nlen_total:
                        nlen = min(512, nlen_total - n0)
                        ps = psp.tile([P, 512], FP32, tag="psA", bufs=4, name="psA")
                        for kt in range(KT):
                            nc.tensor.matmul(
                                ps[:, :nlen],
                                lhsT=w_sb[name][:, kt, ms],
                                rhs=xt[kt][:, n0 : n0 + nlen],
                                start=(kt == 0),
                                stop=(kt == KT - 1),
                            )
                        if name == "q":
                            nc.scalar.activation(
                                dst[mt][:, n0 : n0 + nlen],
                                ps[:, :nlen],
                                AF.Identity,
                                bias=b_col[name][:, mt : mt + 1],
                            )
                        else:
                            nc.vector.tensor_scalar_add(
                                dst[mt][:, n0 : n0 + nlen],
                                ps[:, :nlen],
                                b_col[name][:, mt : mt + 1],
                            )
                        n0 += nlen

            if b == 0:
                # materialized V bias [128, H] fp32 (free-dim bias add on copy)
                for n0, nlen in ((0, 512), (512, 256)):
                    ps = psp.tile([P, 512], FP32, tag="psA", bufs=4, name="psA")
                    nc.tensor.matmul(
                        ps[:, :nlen],
                        lhsT=onesrow[:],
                        rhs=bv_row[0:1, n0 : n0 + nlen],
                        start=True,
                        stop=True,
                    )
                    nc.vector.tensor_copy(bvb[:, n0 : n0 + nlen], ps[:, :nlen])

            # V: out[t-tile, o-chunk] -> vext (bf16, 65-strided) + vterm fp32
            for mt in range(KT):
                ms = slice(mt * P, (mt + 1) * P)
                for n0, nlen in ((0, 512), (512, 256)):
                    ps = psp.tile([P, 512], FP32, tag="psA", bufs=4, name="psA")
                    for kt in range(KT):
                        nc.tensor.matmul(
                            ps[:, :nlen],
                            lhsT=xt[kt][:, ms],
                            rhs=w_sb["v"][:, kt, n0 : n0 + nlen],
                            start=(kt == 0),
                            stop=(kt == KT - 1),
                        )
                    nh0 = n0 // DH
                    nheads = nlen // DH
                    vv = vext[mt].rearrange("p (h c) -> p h c", c=VW)
                    nc.vector.tensor_tensor(
                        out=vv[:, nh0 : nh0 + nheads, 0:DH],
                        in0=ps[:, :nlen].rearrange("p (h c) -> p h c", c=DH),
                        in1=bvb[:, n0 : n0 + nlen].rearrange("p (h c) -> p h c", c=DH),
                        op=ALU.add,
                    )
                    if mt == KT - 1:
                        # fp32 copy of term-value rows for output passthrough
                        nc.scalar.activation(
                            vterm[:, n0 : n0 + nlen],
                            ps[:, :nlen],
                            AF.Copy,
                        )
                vv = vext[mt].rearrange("p (h c) -> p h c", c=VW)
                nc.gpsimd.memset(vv[:, :, DH : DH + 1], 1.0)
            # vterm still needs the bias
            nc.vector.tensor_add(vterm[:], vterm[:], bvb[:])
            # term rows pass through V (fp32, bias included) - DMA out early
            nc.sync.dma_start(out=out[b][NQ:S, :], in_=vterm[:])

            # ---- per-block value sums, stored with 65th col = 64.0 so the
            # notselC correction matmul also contributes 9*64=576 to Z ----
            vsumsE = smp.tile([CDD, NH * VW], BF16, tag="vsums", name="vsumsE")
            for n0 in (0, 384):
                ps = psp.tile([P, 512], FP32, tag="psA", bufs=4, name="psA")
                nh0 = n0 // DH
                for kt in range(5):
                    rhs = vext[kt].rearrange("p (h c) -> p h c", c=VW)[
                        :, nh0 : nh0 + 6, 0:DH
                    ]
                    nc.tensor.matmul(
                        ps[0:CDD, 0:384],
                        lhsT=G[:, 10 - 2 * kt : 20 - 2 * kt],
                        rhs=rhs,
                        start=(kt == 0),
                        stop=(kt == 4),
                    )
                vsv = vsumsE.rearrange("p (h c) -> p h c", c=VW)
                nc.vector.tensor_copy(
                    vsv[:, nh0 : nh0 + 6, 0:DH],
                    ps[0:CDD, 0:384].rearrange("p (h c) -> p h c", c=DH),
                )
            vsv = vsumsE.rearrange("p (h c) -> p h c", c=VW)
            nc.gpsimd.memset(vsv[:, :, DH : DH + 1], float(L))

            # prefetch next batch's X^T while attention runs (SWDGE path)
            if b + 1 < BL:
                xt_next = []
                for kt in range(KT):
                    t = xtp.tile([P, S], BF16, tag=f"xt{kt}", name=f"xt{kt}")
                    nc.sync.dma_start(
                        out=t[:], in_=x[b + 1][kt * P : (kt + 1) * P, :]
                    )
                    xt_next.append(t)

            # ---- attention ----
            def emit_scores(hg):
                se_t = [sep.tile([P, NQ], BF16, tag=f"set{i}", name=f"set{i}") for i in range(HGS)]
                se_b = [sep.tile([P, 5 * L], BF16, tag=f"seb{i}", name=f"seb{i}") for i in range(HGS)]
                for hl in range(HGS):
                    hh = hg * HGS + hl
                    pt, r0 = hh // 2, (hh % 2) * 64
                    QTh = qt_sb[pt][r0 : r0 + 64, :]
                    KTh = kt_sb[pt][r0 : r0 + 64, :]
                    # term scores^T [128 terms, 640 q]
                    for n0 in (0, 320):
                        ps = psp.tile([P, 512], FP32, tag="psA", bufs=4, name="psA")
                        nc.tensor.matmul(
                            ps[:, 0:320],
                            lhsT=KTh[:, NQ:S],
                            rhs=QTh[:, n0 : n0 + 320],
                            start=True,
                            stop=True,
                        )
                        nc.scalar.activation(
                            se_t[hl][:, n0 : n0 + 320],
                            ps[:, 0:320],
                            AF.Exp,
                            scale=0.125,
                        )
                    # block scores^T: all 10 blocks in one psum bank
                    ps = psp.tile([P, 5 * L], FP32, tag="psB", bufs=1, name="psB", padded_shape=[P, 512])
                    for j in range(5):
                        for half in (0, 1):
                            c = 2 * j + half
                            cs = slice(c * L, (c + 1) * L)
                            nc.tensor.matmul(
                                ps[half * 64 : half * 64 + 64, j * L : (j + 1) * L],
                                lhsT=KTh[:, cs],
                                rhs=QTh[:, cs],
                                start=True,
                                stop=True,
                            )
                    nc.scalar.activation(
                        se_b[hl][:],
                        ps[:],
                        AF.Exp,
                        scale=0.125,
                    )
                return se_t, se_b

            def emit_pv(hg, se_t, se_b):
                for j in range(5):
                    psc = psp.tile([P, HGS * VW], FP32, tag="psC", bufs=3, name="psC", padded_shape=[P, 512])
                    hgs_v = slice(hg * HGS * VW, (hg + 1) * HGS * VW)
                    # head 0's full-height terms matmul opens the bank's one
                    # accumulation group; everything else accumulates.
                    for hl in range(HGS):
                        hh = hg * HGS + hl
                        vs = slice(hh * VW, (hh + 1) * VW)
                        nc.tensor.matmul(
                            psc[:, hl * VW : (hl + 1) * VW],
                            lhsT=se_t[hl][:, j * P : (j + 1) * P],
                            rhs=vext[5][:, vs],
                            start=(hl == 0),
                            stop=False,
                        )
                    for half in (0, 1):
                        c = 2 * j + half
                        hs = slice(half * 64, half * 64 + 64)
                        nc.tensor.matmul(
                            psc[hs, :],
                            lhsT=notselC[:, c * L : (c + 1) * L],
                            rhs=vsumsE[:, hgs_v],
                            start=False,
                            stop=False,
                        )
                    for hl in range(HGS):
                        hh = hg * HGS + hl
                        c0 = hl * VW
                        vs = slice(hh * VW, (hh + 1) * VW)
                        for half in (0, 1):
                            hs = slice(half * 64, half * 64 + 64)
                            nc.tensor.matmul(
                                psc[hs, c0 : c0 + VW],
                                lhsT=se_b[hl][hs, j * L : (j + 1) * L],
                                rhs=vext[j][hs, vs],
                                start=False,
                                stop=False,
                            )
                    # full-height +0 rank-1 whose stop closes the bank's group
                    nc.tensor.matmul(
                        psc[:, DH : DH + 1],
                        lhsT=onesrow[:],
                        rhs=zrow[:],
                        start=False,
                        stop=True,
                    )
                    zr = smp.tile([P, HGS], FP32, tag="zr", bufs=4, name="zr")
                    pscv = psc.rearrange("p (h c) -> p h c", c=VW)
                    nc.vector.reciprocal(
                        zr[:].rearrange("p (h o) -> p h o", o=1),
                        pscv[:, :, DH : DH + 1],
                    )
                    ob = osp.tile([P, HGS * DH], FP32, tag=f"osb{j}", bufs=2, name=f"osb{j}")
                    in0 = pscv[:, :, 0:DH]
                    in1 = zr[:].rearrange("p (h o) -> p h o", o=1)
                    bin0, bin1 = bass.broadcast_tensor_aps(in0, in1)
                    nc.vector.tensor_tensor(
                        out=ob[:].rearrange("p (h c) -> p h c", c=DH),
                        in0=bin0,
                        in1=bin1,
                        op=ALU.mult,
                    )
                    nc.sync.dma_start(
                        out=out[b][j * P : (j + 1) * P, hg * HGS * DH : (hg + 1) * HGS * DH],
                        in_=ob[:],
                    )

            prev = None
            for hg in range(NHG):
                cur = emit_scores(hg)
                if prev is not None:
                    emit_pv(hg - 1, *prev)
                prev = cur
            emit_pv(NHG - 1, *prev)

_CACHE = {}


def _get_program():
    if "nc" not in _CACHE:
        _CACHE["nc"] = _build_program()
    return _CACHE["nc"]


def _make_in_maps(inputs):
    hs = np.asarray(inputs["hidden_states"], np.float32)
    hst = np.ascontiguousarray(hs.transpose(0, 2, 1)).astype(ml_dtypes.bfloat16)
    wq = np.asarray(inputs["Wq"], np.float32)
    wk = np.asarray(inputs["Wk"], np.float32)
    wv = np.asarray(inputs["Wv"], np.float32)
    in_common = {
        "wqt": np.ascontiguousarray(wq.T).astype(ml_dtypes.bfloat16),
        "wkt": np.ascontiguousarray(wk.T).astype(ml_dtypes.bfloat16),
        "wvt": np.ascontiguousarray(wv.T).astype(ml_dtypes.bfloat16),
        "bq": np.asarray(inputs["bq"], np.float32),
        "bk": np.asarray(inputs["bk"], np.float32),
        "bv16": np.asarray(inputs["bv"], np.float32).astype(ml_dtypes.bfloat16),
    }
    return [
        {"x": hst[i * BL : (i + 1) * BL], **in_common} for i in range(NCORES)
    ]


def kernel(**inputs) -> np.ndarray:
    in_maps = _make_in_maps(inputs)
    nc = _get_program()
    res = run_bass_kernel_spmd(nc, in_maps, list(range(NCORES)))
    return np.concatenate([res.results[i]["out"] for i in range(NCORES)], axis=0)



# revision 2
# speedup vs baseline: 1.6674x; 1.6674x over previous
"""Trainium2 Bass kernel for nn_BertSelfAttention_79577154060613 (fp8).

Block-sparse BERT self-attention, data-parallel over batch across 8 cores
(2 batches/core). Cost-model-guided redesign of the bf16 baseline:

- All projections run as fp8e4m3 DoubleRow matmuls: 2 contraction k-tiles
  packed per instruction at 0.5 cycles/row -> 4x cheaper than bf16. Weights
  are pre-scaled by 16 on the host so w*16 lands in fp8's normal range; the
  PSUM->SBUF copies divide by 16 (free via the copy's scale port).
- The V projection of the 128 term tokens runs in bf16 (the reference output
  passes those rows through untouched, so they set the error floor). All
  other fp8 error sources only perturb softmax-averaged context and stay
  ~1e-3 absolute.
- Per head, term scores ([128 terms x 640 q]) and block scores land in one
  3-bank PSUM tile; block scores are computed as 5 block-diagonal [128x128]
  key-x-query tiles whose off-diagonal quadrants are zero-filled by two
  rank-1 DoubleRow matmuls. exp(0)=1 garbage in those quadrants contributes
  exactly sum(v over the sibling block), which the correction term absorbs
  by excluding the whole block PAIR instead of just the own block:
    ctx*Z = sum_{k in block|terms} e^s v_k + 1*Vsum_sibling + corr'(c)
    corr'(c) = sum_{c' not in pair(c)} Vsum_c'   (rank-10 matmul vs vext)
  This allows ONE exp instruction per head for all block scores.
- PV is a single DoubleRow matmul per (q-tile, head): contraction half A =
  128 exp'ed term scores, half B = the 128-row block-diagonal exp'ed block
  scores; rhs halves are vext[term-tile] and vext[j-tile]. The softmax
  denominator is accumulated by rank-1 DoubleRow matmuls (se @ 0.25-column)
  into columns [768:780] of the same PSUM pair, with the corr matmul
  contributing the 0.25*512 constant from masked-out keys; the heads' 64-col
  context slices are gapless so the divide is ONE DVE op per q-tile.
- Output is staged in SBUF [128, 6, 768] fp32 and DMA'd per 128-row slice
  (6 DMAs/batch). Phases are software-pipelined: batch0 PV hides inside
  batch1's projection loop; batch1 PV double-buffers via the then-idle
  3-bank PSUM tag.
"""

import numpy as np
import ml_dtypes

import concourse.bass as bass
import concourse.mybir as mybir
import concourse.tile as tile
from concourse import bacc
from concourse.bass_utils import run_bass_kernel_spmd

B, CDD, L, T, H, NH = 16, 10, 64, 128, 768, 12
DH = H // NH  # 64
S = CDD * L + T  # 768
NQ = CDD * L  # 640
P = 128
NCORES = 8
BL = B // NCORES  # 2
KT = H // P  # 6
FP32 = mybir.dt.float32
BF16 = mybir.dt.bfloat16
FP8 = mybir.dt.float8e4
AF = mybir.ActivationFunctionType
ALU = mybir.AluOpType
DR = mybir.MatmulPerfMode.DoubleRow
ONECOL = 0.25  # Z-column scale (keeps the corr Z constant fp8-exact: 128)
WS = 16.0  # host-side weight scale
IWS = 1.0 / WS
NJ = NQ // P  # 5 q-tiles
ZC = NH * DH  # 768: column where the Z region starts in psc


def _build_program():
    nc = bacc.Bacc(
        "TRN2", target_bir_lowering=False, debug=False, num_devices=NCORES
    )
    x8 = nc.dram_tensor("x8", [BL, H, S], FP8, kind="ExternalInput").ap()
    xt16 = nc.dram_tensor("xt16", [BL, P, KT, T], BF16, kind="ExternalInput").ap()
    wq8 = nc.dram_tensor("wq8", [H, H], FP8, kind="ExternalInput").ap()
    wk8 = nc.dram_tensor("wk8", [H, H], FP8, kind="ExternalInput").ap()
    wv8 = nc.dram_tensor("wv8", [H, H], FP8, kind="ExternalInput").ap()
    wv16 = nc.dram_tensor("wv16", [H, H], BF16, kind="ExternalInput").ap()
    bq = nc.dram_tensor("bq", [H], FP32, kind="ExternalInput").ap()
    bk = nc.dram_tensor("bk", [H], FP32, kind="ExternalInput").ap()
    bv16 = nc.dram_tensor("bv16", [H], BF16, kind="ExternalInput").ap()
    out = nc.dram_tensor("out", [BL, S, H], FP32, kind="ExternalOutput").ap()

    with tile.TileContext(nc) as tc:
        _emit(tc, nc, x8, xt16, wq8, wk8, wv8, wv16, bq, bk, bv16, out)
    nc.compile()
    return nc


def _emit(tc, nc, x8, xt16, wq8, wk8, wv8, wv16, bq, bk, bv16, out):
    from contextlib import ExitStack

    ctx = ExitStack()
    with ctx:
        cp = ctx.enter_context(tc.tile_pool(name="consts", bufs=1))
        wp = ctx.enter_context(tc.tile_pool(name="weights", bufs=1))
        xp = ctx.enter_context(tc.tile_pool(name="xin", bufs=2))
        qp = ctx.enter_context(tc.tile_pool(name="qkv", bufs=2))
        sp = ctx.enter_context(tc.tile_pool(name="sexp", bufs=2))
        op = ctx.enter_context(tc.tile_pool(name="ostg", bufs=2))
        mp = ctx.enter_context(tc.tile_pool(name="small", bufs=2))
        pa = ctx.enter_context(tc.tile_pool(name="ps", bufs=1, space="PSUM"))

        def pjt(name):
            return pa.tile(
                [P, 1536], FP32, tag="pj", bufs=2, name=name,
                padded_shape=[P, 1536],
            )

        def smt(name):
            return pa.tile(
                [P, 1024], FP32, tag="sm", bufs=1, name=name,
                padded_shape=[P, 1024],
            )

        # ---------------- input DMAs (critical-path order) ----------------
        # first Q m-tile needs wq cols [0:128] and xt kt-pair 0 only: split
        # those DMAs so the PE can start ~1.5us in
        wq_sb = wp.tile([P, KT, H], FP8, name="wq8sb")
        wq_r = wq8.rearrange("(k p) o -> p k o", p=P)
        nc.sync.dma_start(out=wq_sb[:, :, 0:P], in_=wq_r[:, :, 0:P])
        xt_t, xm_t = [], []
        for b in range(BL):
            xt_t.append(xp.tile([P, KT, S], FP8, tag="xt", name=f"xt{b}"))
            xm_t.append(xp.tile([P, KT, T], BF16, tag="xm", name=f"xm{b}"))
        x0_r = x8[0].rearrange("(k p) s -> p k s", p=P)
        nc.sync.dma_start(out=xt_t[0][:, 0:2, :], in_=x0_r[:, 0:2, :])
        bcq = cp.tile([P, KT], FP32, name="bcq")
        nc.sync.dma_start(out=bcq[:], in_=bq.rearrange("(t p) -> p t", p=P))
        bck = cp.tile([P, KT], FP32, name="bck")
        nc.sync.dma_start(out=bck[:], in_=bk.rearrange("(t p) -> p t", p=P))
        nc.sync.dma_start(out=xt_t[0][:, 2:4, :], in_=x0_r[:, 2:4, :])
        nc.sync.dma_start(out=xt_t[0][:, 4:6, :], in_=x0_r[:, 4:6, :])
        wk_sb = wp.tile([P, KT, H], FP8, name="wk8sb")
        wk_r = wk8.rearrange("(k p) o -> p k o", p=P)
        nc.sync.dma_start(out=wk_sb[:, :, 0:P], in_=wk_r[:, :, 0:P])
        nc.sync.dma_start(out=wq_sb[:, :, P:H], in_=wq_r[:, :, P:H])
        nc.sync.dma_start(out=wk_sb[:, :, P:H], in_=wk_r[:, :, P:H])
        wv_sb = wp.tile([P, KT, H], FP8, name="wv8sb")
        nc.sync.dma_start(out=wv_sb[:], in_=wv8.rearrange("(k p) o -> p k o", p=P))
        bvrow = cp.tile([1, H], BF16, name="bvrow")
        nc.sync.dma_start(out=bvrow[:], in_=bv16[None, :])
        nc.sync.dma_start(out=xt_t[1][:], in_=x8[1].rearrange("(k p) s -> p k s", p=P))
        wv16_sb = wp.tile([P, KT, H], BF16, name="wv16sb")
        nc.sync.dma_start(out=wv16_sb[:], in_=wv16.rearrange("(k p) o -> p k o", p=P))
        nc.sync.dma_start(out=xm_t[0][:], in_=xt16[0])
        nc.sync.dma_start(out=xm_t[1][:], in_=xt16[1])

        # ---------------- constants (Pool) ----------------
        onesrow = cp.tile([1, P], FP8, name="onesrow")
        nc.gpsimd.memset(onesrow[:], 1.0)
        zpair = cp.tile([1, 2, P], FP8, name="zpair")
        nc.gpsimd.memset(zpair[:], 0.0)
        onecol2 = cp.tile([P, 2, 1], FP8, name="onecol2")
        nc.gpsimd.memset(onecol2[:], ONECOL)
        # notG6[p, kt, c] = 0 if block c is in tile kt's pair else 1.
        # Inner dim padded to 64 so dual-fp8 LdWeights half-stride is aligned
        # (cols 10:64 are zero -> psum rows 10:64 unused).
        notG6 = cp.tile([P, KT, 64], FP8, name="notG6")
        nc.gpsimd.memset(notG6[:], 0.0)
        nc.gpsimd.memset(notG6[:, :, 0:CDD], 1.0)
        for kt in range(5):
            nc.gpsimd.memset(notG6[:, kt, 2 * kt : 2 * kt + 2], 0.0)
        # indall[c, j, q] = 1 iff query q of tile j belongs to block c,
        # i.e. c - 2j - (q // 64) == 0
        indall = cp.tile([CDD, NJ, P], FP8, name="indall")
        nc.gpsimd.memset(indall[:], 1.0)
        nc.gpsimd.affine_select(
            out=indall.rearrange("c j (h q) -> c j h q", q=64),
            in_=indall.rearrange("c j (h q) -> c j h q", q=64),
            compare_op=ALU.is_equal,
            fill=0.0,
            base=0,
            pattern=[[-2, NJ], [-1, 2], [0, 64]],
            channel_multiplier=1,
        )
        bvb = cp.tile([P, H], FP32, name="bvb")

        qt_t = [None] * BL
        kt_t = [None] * BL
        ve_t = [None] * BL
        se_t = [None] * BL
        ce_t = [None] * BL
        sg_t = [None] * BL

        wq8_z = None  # zero-fill rhs uses xt slices (DMA'd first)

        def alloc_bufs(b):
            qt_t[b] = qp.tile([P, KT, NQ], FP8, tag="qt", name=f"qt8_{b}")
            kt_t[b] = qp.tile([P, KT, S], FP8, tag="kt", name=f"kt8_{b}")
            ve_t[b] = qp.tile([P, KT, NH, DH], FP8, tag="ve", name=f"vext_{b}")
            se_t[b] = sp.tile([P, NH, NJ, 2, P], FP8, tag="se", name=f"se_{b}")
            sg_t[b] = op.tile([P, KT, H], FP32, tag="stg", name=f"stg_{b}")

        def qk_mt(b, mt):
            """Q and K projections for one m-tile, sharing a 3-bank psum:
            Q at [0:512]+[512:640], K at [640:1024]+[1024:1408]."""
            ms = slice(mt * P, (mt + 1) * P)
            ps = pjt("psqk")
            for c0, nlen in ((0, 512), (512, 128)):
                for kp in range(3):
                    nc.tensor.matmul(
                        ps[:, c0 : c0 + nlen],
                        lhsT=wq_sb[:, 2 * kp : 2 * kp + 2, ms],
                        rhs=xt_t[b][:, 2 * kp : 2 * kp + 2, c0 : c0 + nlen],
                        start=(kp == 0),
                        stop=(kp == 2),
                        perf_mode=DR,
                    )
            for c0, nlen in ((640, 384), (1024, 384)):
                for kp in range(3):
                    nc.tensor.matmul(
                        ps[:, c0 : c0 + nlen],
                        lhsT=wk_sb[:, 2 * kp : 2 * kp + 2, ms],
                        rhs=xt_t[b][:, 2 * kp : 2 * kp + 2, c0 - 640 : c0 - 640 + nlen],
                        start=(kp == 0),
                        stop=(kp == 2),
                        perf_mode=DR,
                    )
            if mt < 5 or b == 1:  # balance: most Q copies on ACT
                nc.scalar.activation(
                    qt_t[b][:, mt, :], ps[:, 0:NQ], AF.Identity,
                    bias=bcq[:, mt : mt + 1], scale=IWS,
                )
            else:
                nc.vector.tensor_scalar(
                    qt_t[b][:, mt, :], ps[:, 0:NQ], IWS, bcq[:, mt : mt + 1],
                    op0=ALU.mult, op1=ALU.add,
                )
            nc.vector.tensor_scalar(
                kt_t[b][:, mt, :], ps[:, 640 : 640 + S], IWS, bck[:, mt : mt + 1],
                op0=ALU.mult, op1=ALU.add,
            )

        def v_tt(b, tt):
            """Candidate-token V tile (fp8)."""
            ts = slice(tt * P, (tt + 1) * P)
            psv = smt("psv")
            for c0, nlen in ((0, 512), (512, 256)):
                for kp in range(3):
                    nc.tensor.matmul(
                        psv[:, c0 : c0 + nlen],
                        lhsT=xt_t[b][:, 2 * kp : 2 * kp + 2, ts],
                        rhs=wv_sb[:, 2 * kp : 2 * kp + 2, c0 : c0 + nlen],
                        start=(kp == 0),
                        stop=(kp == 2),
                        perf_mode=DR,
                    )
            nc.vector.scalar_tensor_tensor(
                out=ve_t[b][:, tt, :, :],
                in0=psv[:, 0:H].rearrange("p (h c) -> p h c", c=DH),
                scalar=IWS,
                in1=bvb.rearrange("p (h c) -> p h c", c=DH),
                op0=ALU.mult,
                op1=ALU.add,
            )

        def v_term(b):
            """Term-token V in bf16 (output passthrough accuracy)."""
            psv = smt("psvt")
            for c0, nlen in ((0, 512), (512, 256)):
                for kt in range(KT):
                    nc.tensor.matmul(
                        psv[:, c0 : c0 + nlen],
                        lhsT=xm_t[b][:, kt, :],
                        rhs=wv16_sb[:, kt, c0 : c0 + nlen],
                        start=(kt == 0),
                        stop=(kt == KT - 1),
                    )
            stg = sg_t[b]
            nc.vector.tensor_tensor(
                out=stg[:, 5, :], in0=psv[:, 0:H], in1=bvb[:], op=ALU.add
            )
            nc.gpsimd.tensor_copy(
                ve_t[b][:, 5, :, :],
                stg[:, 5, :].rearrange("p (h c) -> p h c", c=DH),
            )
            nc.sync.dma_start(
                out=out[b].rearrange("(r p) h -> p r h", p=P)[:, 5, :],
                in_=stg[:, 5, :],
            )

        def scores_h(b, hh):
            mt, hl = hh // 2, hh % 2
            r0 = hl * 64
            KTh = kt_t[b][r0 : r0 + 64, mt, :]
            QTh = qt_t[b][r0 : r0 + 64, mt, :]
            se = se_t[b]
            ph = pjt("ph")
            # se half 0 = block scores (pairs with vext[j] in the PV DR
            # matmul), half 1 = term scores (pairs with vext[5]).
            # zero-fill block region [0:640] (banks 0-1)
            nc.tensor.matmul(
                ph[:, 0:512], lhsT=zpair[:], rhs=xt_t[b][0:1, 0:2, 0:512],
                start=True, stop=False, perf_mode=DR,
            )
            nc.tensor.matmul(
                ph[:, 512:640], lhsT=zpair[:], rhs=xt_t[b][0:1, 0:2, 0:128],
                start=True, stop=False, perf_mode=DR,
            )
            # block-diagonal scores: per q-tile j a [128k x 128q] tile
            for j in range(NJ):
                for half in range(2):
                    c = 2 * j + half
                    nc.tensor.matmul(
                        ph[
                            half * 64 : half * 64 + 64,
                            j * P + half * 64 : j * P + half * 64 + 64,
                        ],
                        lhsT=KTh[:, c * L : (c + 1) * L],
                        rhs=QTh[:, c * L : (c + 1) * L],
                        start=False,
                        stop=(half == 1 and j in (3, 4)),
                    )
            # term scores^T [128 terms, 640 q] in cols [640:1280]
            nc.tensor.matmul(
                ph[:, 640:1024], lhsT=KTh[:, NQ:S], rhs=QTh[:, 0:384],
                start=True, stop=True,
            )
            nc.tensor.matmul(
                ph[:, 1024:1280], lhsT=KTh[:, NQ:S], rhs=QTh[:, 384:640],
                start=True, stop=True,
            )
            # exp (ACT): terms + blocks in ONE instruction (both APs uniform)
            nc.scalar.activation(
                se[:, hh, :, :, :].rearrange("p j two q -> p two j q"),
                ph[:, 0:1280].rearrange("p (two j q) -> p two j q", j=NJ, q=P),
                AF.Exp, scale=0.125,
            )

        def vsums(b):
            """corr'[c] = sum of v over candidate tokens NOT in pair(c)."""
            vef = ve_t[b].rearrange("p k h c -> p k (h c)")
            psc = smt("pscor")
            for c0, nlen in ((0, 512), (512, 256)):
                cs = slice(c0, c0 + nlen)
                for kp in range(2):
                    nc.tensor.matmul(
                        psc[0:64, cs],
                        lhsT=notG6[:, 2 * kp : 2 * kp + 2, :],
                        rhs=vef[:, 2 * kp : 2 * kp + 2, cs],
                        start=(kp == 0),
                        stop=False,
                        perf_mode=DR,
                    )
                nc.tensor.matmul(
                    psc[0:CDD, cs],
                    lhsT=notG6[:, 4, 0:CDD],
                    rhs=vef[:, 4, cs],
                    start=False,
                    stop=True,
                )
            corrE = mp.tile([CDD, ZC + NH], FP8, tag="corr", name=f"corrE_{b}")
            ce_t[b] = corrE
            nc.vector.tensor_copy(corrE[:, 0:ZC], psc[0:CDD, 0:ZC])
            # Z constant: 0.25 * 512 masked-out keys, via indall (0/1 rows)
            nc.gpsimd.memset(corrE[:, ZC : ZC + NH], 128.0)

        def pv_j(b, j, tag, last=False):
            se, vext, corrE, stg = se_t[b], ve_t[b], ce_t[b], sg_t[b]
            psc = pjt("pspv") if tag == "pj" else smt("pspv")
            # corr opens both banks' accumulation groups
            nc.tensor.matmul(
                psc[:, 0:512], lhsT=indall[:, j, :], rhs=corrE[:, 0:512],
                start=True, stop=False,
            )
            nc.tensor.matmul(
                psc[:, 512:768], lhsT=indall[:, j, :], rhs=corrE[:, 512:768],
                start=True, stop=False,
            )
            nc.tensor.matmul(
                psc[:, ZC : ZC + NH], lhsT=indall[:, j, :],
                rhs=corrE[:, ZC : ZC + NH],
                start=False, stop=False,
            )
            for hh in range(NH):
                nc.tensor.matmul(
                    psc[:, hh * DH : (hh + 1) * DH],
                    lhsT=se[:, hh, j, :, :],
                    rhs=vext[:, j : KT : 5 - j, hh, :],
                    start=False,
                    stop=(hh == 7),
                    perf_mode=DR,
                )
                nc.tensor.matmul(
                    psc[:, ZC + hh : ZC + hh + 1],
                    lhsT=se[:, hh, j, :, :],
                    rhs=onecol2[:],
                    start=False,
                    stop=(hh == NH - 1),
                    perf_mode=DR,
                )
            zr = mp.tile([P, NH], FP32, tag="zr", bufs=2, name="zr")
            nc.vector.reciprocal(zr[:], psc[:, ZC : ZC + NH])
            halves = ((0, 6), (6, 12)) if last else ((0, 12),)
            for lo, hi in halves:
                in0 = psc[:, lo * DH : hi * DH].rearrange(
                    "p (h c) -> p h c", c=DH
                )
                in1 = zr[:, lo:hi].rearrange("p (h o) -> p h o", o=1)
                bin0, bin1 = bass.broadcast_tensor_aps(in0, in1)
                nc.vector.scalar_tensor_tensor(
                    out=stg[:, j, lo * DH : hi * DH].rearrange(
                        "p (h c) -> p h c", c=DH
                    ),
                    in0=bin0,
                    scalar=ONECOL,
                    in1=bin1,
                    op0=ALU.mult,
                    op1=ALU.mult,
                )
                nc.sync.dma_start(
                    out=out[b].rearrange("(r p) h -> p r h", p=P)[
                        :, j, lo * DH : hi * DH
                    ],
                    in_=stg[:, j, lo * DH : hi * DH],
                )

        # ---------------- schedule ----------------
        # Scores lag projections by one m-tile so each group's score matmuls
        # overlap the NEXT group's projections instead of waiting on their
        # own Q/K copies. Batch 0 PV hides inside batch 1's loop.
        alloc_bufs(0)
        alloc_bufs(1)

        for mt in range(KT):
            qk_mt(0, mt)
            if mt == 0:
                # bvb[p, o] = bv[o] broadcast (rank-1); after first group so
                # the PE isn't blocked on the bvrow DMA at t=0
                psb = smt("psbv")
                nc.tensor.matmul(psb[:, 0:512], lhsT=onesrow[:], rhs=bvrow[0:1, 0:512], start=True, stop=True)
                nc.tensor.matmul(psb[:, 512:768], lhsT=onesrow[:], rhs=bvrow[0:1, 512:768], start=True, stop=True)
                nc.vector.tensor_copy(bvb[:, 0:512], psb[:, 0:512])
                nc.vector.tensor_copy(bvb[:, 512:768], psb[:, 512:768])
            else:
                v_tt(0, mt - 1)
                scores_h(0, 2 * (mt - 1))
                scores_h(0, 2 * (mt - 1) + 1)
        v_tt(0, 4)
        v_term(0)
        vsums(0)
        scores_h(0, 10)
        scores_h(0, 11)

        for mt in range(KT):
            qk_mt(1, mt)
            if mt < 5:
                v_tt(1, mt)
            if mt >= 1:
                scores_h(1, 2 * (mt - 1))
                scores_h(1, 2 * (mt - 1) + 1)
                pv_j(0, mt - 1, "pv")
        v_term(1)
        vsums(1)
        scores_h(1, 10)
        scores_h(1, 11)
        pv_j(0, 4, "pv")

        # batch 1 PV, double-buffered via the now-idle 3-bank pj tag
        for j in range(NJ):
            pv_j(1, j, "pj", last=(j == NJ - 1))


_CACHE = {}


def _get_program():
    if "nc" not in _CACHE:
        _CACHE["nc"] = _build_program()
    return _CACHE["nc"]


def _make_in_maps(inputs):
    f8 = ml_dtypes.float8_e4m3
    bf = ml_dtypes.bfloat16
    hs = np.asarray(inputs["hidden_states"], np.float32)
    hst = np.ascontiguousarray(hs.transpose(0, 2, 1))  # [B, H, S]
    wq = np.asarray(inputs["Wq"], np.float32).T * WS
    wk = np.asarray(inputs["Wk"], np.float32).T * WS
    wv = np.asarray(inputs["Wv"], np.float32).T
    in_common = {
        "wq8": np.ascontiguousarray(wq).astype(f8),
        "wk8": np.ascontiguousarray(wk).astype(f8),
        "wv8": np.ascontiguousarray(wv * WS).astype(f8),
        "wv16": np.ascontiguousarray(wv).astype(bf),
        "bq": np.asarray(inputs["bq"], np.float32),
        "bk": np.asarray(inputs["bk"], np.float32),
        "bv16": np.asarray(inputs["bv"], np.float32).astype(bf),
    }
    x8 = hst.astype(f8)
    # xt16 packed [BL, P, KT, T] so the DMA's contiguous runs are 1536B
    xm = (
        hst[:, :, NQ:]
        .reshape(B, KT, P, T)
        .transpose(0, 2, 1, 3)
        .astype(bf)
    )
    return [
        {
            "x8": x8[i * BL : (i + 1) * BL],
            "xt16": np.ascontiguousarray(xm[i * BL : (i + 1) * BL]),
            **in_common,
        }
        for i in range(NCORES)
    ]


def kernel(**inputs) -> np.ndarray:
    in_maps = _make_in_maps(inputs)
    nc = _get_program()
    res = run_bass_kernel_spmd(nc, in_maps, list(range(NCORES)))
    return np.concatenate([res.results[i]["out"] for i in range(NCORES)], axis=0)


# revision 3
# speedup vs baseline: 1.6935x; 1.0156x over previous
"""Trainium2 Bass kernel for nn_BertSelfAttention_79577154060613 (fp8).

Block-sparse BERT self-attention, data-parallel over batch across 8 cores
(2 batches/core). Cost-model-guided redesign of the bf16 baseline:

- All projections run as fp8e4m3 DoubleRow matmuls: 2 contraction k-tiles
  packed per instruction at 0.5 cycles/row -> 4x cheaper than bf16. Weights
  are pre-scaled by 16 on the host so w*16 lands in fp8's normal range; the
  PSUM->SBUF copies divide by 16 (free via the copy's scale port).
- The V projection of the 128 term tokens runs in bf16 (the reference output
  passes those rows through untouched, so they set the error floor). All
  other fp8 error sources only perturb softmax-averaged context and stay
  ~1e-3 absolute.
- Per head, term scores ([128 terms x 640 q]) and block scores land in one
  3-bank PSUM tile; block scores are computed as 5 block-diagonal [128x128]
  key-x-query tiles whose off-diagonal quadrants are zero-filled by two
  rank-1 DoubleRow matmuls. exp(0)=1 garbage in those quadrants contributes
  exactly sum(v over the sibling block), which the correction term absorbs
  by excluding the whole block PAIR instead of just the own block:
    ctx*Z = sum_{k in block|terms} e^s v_k + 1*Vsum_sibling + corr'(c)
    corr'(c) = sum_{c' not in pair(c)} Vsum_c'   (rank-10 matmul vs vext)
  This allows ONE exp instruction per head for all block scores.
- PV is a single DoubleRow matmul per (q-tile, head): contraction half A =
  128 exp'ed term scores, half B = the 128-row block-diagonal exp'ed block
  scores; rhs halves are vext[term-tile] and vext[j-tile]. The softmax
  denominator is accumulated by rank-1 DoubleRow matmuls (se @ 0.25-column)
  into columns [768:780] of the same PSUM pair, with the corr matmul
  contributing the 0.25*512 constant from masked-out keys; the heads' 64-col
  context slices are gapless so the divide is ONE DVE op per q-tile.
- Output is staged in SBUF [128, 6, 768] fp32 and DMA'd per 128-row slice
  (6 DMAs/batch). Phases are software-pipelined: batch0 PV hides inside
  batch1's projection loop; batch1 PV double-buffers via the then-idle
  3-bank PSUM tag.
"""

import numpy as np
import ml_dtypes

import concourse.bass as bass
import concourse.mybir as mybir
import concourse.tile as tile
from concourse import bacc
from concourse.bass_utils import run_bass_kernel_spmd

B, CDD, L, T, H, NH = 16, 10, 64, 128, 768, 12
DH = H // NH  # 64
S = CDD * L + T  # 768
NQ = CDD * L  # 640
P = 128
NCORES = 8
BL = B // NCORES  # 2
KT = H // P  # 6
FP32 = mybir.dt.float32
BF16 = mybir.dt.bfloat16
FP8 = mybir.dt.float8e4
AF = mybir.ActivationFunctionType
ALU = mybir.AluOpType
DR = mybir.MatmulPerfMode.DoubleRow
ONECOL = 0.25  # Z-column scale (keeps the corr Z constant fp8-exact: 128)
WS = 16.0  # host-side weight scale
IWS = 1.0 / WS
NJ = NQ // P  # 5 q-tiles
ZC = NH * DH  # 768: column where the Z region starts in psc


def _build_program():
    nc = bacc.Bacc(
        "TRN2", target_bir_lowering=False, debug=False, num_devices=NCORES
    )
    x8 = nc.dram_tensor("x8", [BL, H, S], FP8, kind="ExternalInput").ap()
    xt16 = nc.dram_tensor("xt16", [BL, P, KT, T], BF16, kind="ExternalInput").ap()
    wq8 = nc.dram_tensor("wq8", [H, H], FP8, kind="ExternalInput").ap()
    wk8 = nc.dram_tensor("wk8", [H, H], FP8, kind="ExternalInput").ap()
    wv8 = nc.dram_tensor("wv8", [H, H], FP8, kind="ExternalInput").ap()
    wv16 = nc.dram_tensor("wv16", [H, H], BF16, kind="ExternalInput").ap()
    bq = nc.dram_tensor("bq", [H], FP32, kind="ExternalInput").ap()
    bk = nc.dram_tensor("bk", [H], FP32, kind="ExternalInput").ap()
    bv16 = nc.dram_tensor("bv16", [H], BF16, kind="ExternalInput").ap()
    out = nc.dram_tensor("out", [BL, S, H], FP32, kind="ExternalOutput").ap()

    with tile.TileContext(nc) as tc:
        _emit(tc, nc, x8, xt16, wq8, wk8, wv8, wv16, bq, bk, bv16, out)
    nc.compile()
    return nc


def _emit(tc, nc, x8, xt16, wq8, wk8, wv8, wv16, bq, bk, bv16, out):
    from contextlib import ExitStack

    ctx = ExitStack()
    with ctx:
        cp = ctx.enter_context(tc.tile_pool(name="consts", bufs=1))
        wp = ctx.enter_context(tc.tile_pool(name="weights", bufs=1))
        xp = ctx.enter_context(tc.tile_pool(name="xin", bufs=2))
        qp = ctx.enter_context(tc.tile_pool(name="qkv", bufs=2))
        sp = ctx.enter_context(tc.tile_pool(name="sexp", bufs=2))
        op = ctx.enter_context(tc.tile_pool(name="ostg", bufs=2))
        mp = ctx.enter_context(tc.tile_pool(name="small", bufs=2))
        pa = ctx.enter_context(tc.tile_pool(name="ps", bufs=1, space="PSUM"))

        def pjt(name):
            return pa.tile(
                [P, 1536], FP32, tag="pj", bufs=2, name=name,
                padded_shape=[P, 1536],
            )

        def smt(name):
            return pa.tile(
                [P, 1024], FP32, tag="sm", bufs=1, name=name,
                padded_shape=[P, 1024],
            )

        # ---------------- input DMAs (critical-path order) ----------------
        # first Q m-tile needs wq cols [0:128] and xt kt-pair 0 only: split
        # those DMAs so the PE can start ~1.5us in
        wq_sb = wp.tile([P, KT, H], FP8, name="wq8sb")
        wq_r = wq8.rearrange("(k p) o -> p k o", p=P)
        nc.sync.dma_start(out=wq_sb[:, :, 0:P], in_=wq_r[:, :, 0:P])
        xt_t, xm_t = [], []
        for b in range(BL):
            xt_t.append(xp.tile([P, KT, S], FP8, tag="xt", name=f"xt{b}"))
            xm_t.append(xp.tile([P, KT, T], BF16, tag="xm", name=f"xm{b}"))
        x0_r = x8[0].rearrange("(k p) s -> p k s", p=P)
        nc.sync.dma_start(out=xt_t[0][:, 0:2, :], in_=x0_r[:, 0:2, :])
        bcq = cp.tile([P, KT], FP32, name="bcq")
        nc.sync.dma_start(out=bcq[:], in_=bq.rearrange("(t p) -> p t", p=P))
        bck = cp.tile([P, KT], FP32, name="bck")
        nc.sync.dma_start(out=bck[:], in_=bk.rearrange("(t p) -> p t", p=P))
        nc.sync.dma_start(out=xt_t[0][:, 2:4, :], in_=x0_r[:, 2:4, :])
        nc.sync.dma_start(out=xt_t[0][:, 4:6, :], in_=x0_r[:, 4:6, :])
        wk_sb = wp.tile([P, KT, H], FP8, name="wk8sb")
        wk_r = wk8.rearrange("(k p) o -> p k o", p=P)
        nc.sync.dma_start(out=wk_sb[:, :, 0:P], in_=wk_r[:, :, 0:P])
        nc.sync.dma_start(out=wq_sb[:, :, P:H], in_=wq_r[:, :, P:H])
        nc.sync.dma_start(out=wk_sb[:, :, P:H], in_=wk_r[:, :, P:H])
        wv_sb = wp.tile([P, KT, H], FP8, name="wv8sb")
        nc.sync.dma_start(out=wv_sb[:], in_=wv8.rearrange("(k p) o -> p k o", p=P))
        bvrow = cp.tile([1, H], BF16, name="bvrow")
        nc.sync.dma_start(out=bvrow[:], in_=bv16[None, :])
        nc.sync.dma_start(out=xt_t[1][:], in_=x8[1].rearrange("(k p) s -> p k s", p=P))
        wv16_sb = wp.tile([P, KT, H], BF16, name="wv16sb")
        nc.sync.dma_start(out=wv16_sb[:], in_=wv16.rearrange("(k p) o -> p k o", p=P))
        nc.sync.dma_start(out=xm_t[0][:], in_=xt16[0])
        nc.sync.dma_start(out=xm_t[1][:], in_=xt16[1])

        # ---------------- constants (Pool) ----------------
        onesrow = cp.tile([1, P], FP8, name="onesrow")
        nc.gpsimd.memset(onesrow[:], 1.0)
        zpair = cp.tile([1, 2, P], FP8, name="zpair")
        nc.gpsimd.memset(zpair[:], 0.0)
        onecol2 = cp.tile([P, 2, 1], FP8, name="onecol2")
        nc.gpsimd.memset(onecol2[:], ONECOL)
        # notG6[p, kt, c] = 0 if block c is in tile kt's pair else 1.
        # Inner dim padded to 64 so dual-fp8 LdWeights half-stride is aligned
        # (cols 10:64 are zero -> psum rows 10:64 unused).
        notG6 = cp.tile([P, KT, 64], FP8, name="notG6")
        nc.gpsimd.memset(notG6[:], 0.0)
        nc.gpsimd.memset(notG6[:, :, 0:CDD], 1.0)
        for kt in range(5):
            nc.gpsimd.memset(notG6[:, kt, 2 * kt : 2 * kt + 2], 0.0)
        # indall[c, j, q] = 1 iff query q of tile j belongs to block c,
        # i.e. c - 2j - (q // 64) == 0
        indall = cp.tile([CDD, NJ, P], FP8, name="indall")
        nc.gpsimd.memset(indall[:], 1.0)
        nc.gpsimd.affine_select(
            out=indall.rearrange("c j (h q) -> c j h q", q=64),
            in_=indall.rearrange("c j (h q) -> c j h q", q=64),
            compare_op=ALU.is_equal,
            fill=0.0,
            base=0,
            pattern=[[-2, NJ], [-1, 2], [0, 64]],
            channel_multiplier=1,
        )
        bvb = cp.tile([P, H], FP32, name="bvb")

        qt_t = [None] * BL
        kt_t = [None] * BL
        ve_t = [None] * BL
        se_t = [None] * BL
        ce_t = [None] * BL
        sg_t = [None] * BL

        wq8_z = None  # zero-fill rhs uses xt slices (DMA'd first)

        def alloc_bufs(b):
            qt_t[b] = qp.tile([P, KT, NQ], FP8, tag="qt", name=f"qt8_{b}")
            kt_t[b] = qp.tile([P, KT, S], FP8, tag="kt", name=f"kt8_{b}")
            ve_t[b] = qp.tile([P, KT, NH, DH], FP8, tag="ve", name=f"vext_{b}")
            se_t[b] = sp.tile([P, NH, NJ, 2, P], FP8, tag="se", name=f"se_{b}")
            sg_t[b] = op.tile([P, KT, H], FP32, tag="stg", name=f"stg_{b}")

        def qk_mt(b, mt):
            """Q and K projections for one m-tile, sharing a 3-bank psum:
            Q at [0:512]+[512:640], K at [640:1024]+[1024:1408]."""
            ms = slice(mt * P, (mt + 1) * P)
            ps = pjt("psqk")
            # kp-major order for the very first group: each kp slice of x
            # arrives in its own DMA chunk, so the PE starts on chunk 0
            chunks_q = ((0, 512), (512, 128))
            chunks_k = ((640, 384), (1024, 384))
            if b == 0 and mt == 0:
                order = [(kp, c0, nlen, w) for kp in range(3)
                         for (c0, nlen, w) in
                         [c + ("q",) for c in chunks_q] + [c + ("k",) for c in chunks_k]]
            else:
                order = [(kp, c0, nlen, w)
                         for (c0, nlen, w) in
                         [c + ("q",) for c in chunks_q] + [c + ("k",) for c in chunks_k]
                         for kp in range(3)]
            for kp, c0, nlen, w in order:
                wsb = wq_sb if w == "q" else wk_sb
                x0 = c0 if w == "q" else c0 - 640
                nc.tensor.matmul(
                    ps[:, c0 : c0 + nlen],
                    lhsT=wsb[:, 2 * kp : 2 * kp + 2, ms],
                    rhs=xt_t[b][:, 2 * kp : 2 * kp + 2, x0 : x0 + nlen],
                    start=(kp == 0),
                    stop=(kp == 2),
                    perf_mode=DR,
                )
            if mt < 5 or b == 1:  # balance: most Q copies on ACT
                nc.scalar.activation(
                    qt_t[b][:, mt, :], ps[:, 0:NQ], AF.Identity,
                    bias=bcq[:, mt : mt + 1], scale=IWS,
                )
            else:
                nc.vector.tensor_scalar(
                    qt_t[b][:, mt, :], ps[:, 0:NQ], IWS, bcq[:, mt : mt + 1],
                    op0=ALU.mult, op1=ALU.add,
                )
            nc.vector.tensor_scalar(
                kt_t[b][:, mt, :], ps[:, 640 : 640 + S], IWS, bck[:, mt : mt + 1],
                op0=ALU.mult, op1=ALU.add,
            )

        def v_tt(b, tt):
            """Candidate-token V tile (fp8)."""
            ts = slice(tt * P, (tt + 1) * P)
            psv = smt("psv")
            for c0, nlen in ((0, 512), (512, 256)):
                for kp in range(3):
                    nc.tensor.matmul(
                        psv[:, c0 : c0 + nlen],
                        lhsT=xt_t[b][:, 2 * kp : 2 * kp + 2, ts],
                        rhs=wv_sb[:, 2 * kp : 2 * kp + 2, c0 : c0 + nlen],
                        start=(kp == 0),
                        stop=(kp == 2),
                        perf_mode=DR,
                    )
            nc.vector.scalar_tensor_tensor(
                out=ve_t[b][:, tt, :, :],
                in0=psv[:, 0:H].rearrange("p (h c) -> p h c", c=DH),
                scalar=IWS,
                in1=bvb.rearrange("p (h c) -> p h c", c=DH),
                op0=ALU.mult,
                op1=ALU.add,
            )

        def v_term(b):
            """Term-token V in bf16 (output passthrough accuracy)."""
            psv = smt("psvt")
            for c0, nlen in ((0, 512), (512, 256)):
                for kt in range(KT):
                    nc.tensor.matmul(
                        psv[:, c0 : c0 + nlen],
                        lhsT=xm_t[b][:, kt, :],
                        rhs=wv16_sb[:, kt, c0 : c0 + nlen],
                        start=(kt == 0),
                        stop=(kt == KT - 1),
                    )
            stg = sg_t[b]
            nc.vector.tensor_tensor(
                out=stg[:, 5, :], in0=psv[:, 0:H], in1=bvb[:], op=ALU.add
            )
            nc.gpsimd.tensor_copy(
                ve_t[b][:, 5, :, :],
                stg[:, 5, :].rearrange("p (h c) -> p h c", c=DH),
            )
            nc.sync.dma_start(
                out=out[b].rearrange("(r p) h -> p r h", p=P)[:, 5, :],
                in_=stg[:, 5, :],
            )

        def scores_h(b, hh):
            mt, hl = hh // 2, hh % 2
            r0 = hl * 64
            KTh = kt_t[b][r0 : r0 + 64, mt, :]
            QTh = qt_t[b][r0 : r0 + 64, mt, :]
            se = se_t[b]
            ph = pjt("ph")
            # se half 0 = block scores (pairs with vext[j] in the PV DR
            # matmul), half 1 = term scores (pairs with vext[5]).
            # zero-fill block region [0:640] (banks 0-1)
            nc.tensor.matmul(
                ph[:, 0:512], lhsT=zpair[:], rhs=xt_t[b][0:1, 0:2, 0:512],
                start=True, stop=False, perf_mode=DR,
            )
            nc.tensor.matmul(
                ph[:, 512:640], lhsT=zpair[:], rhs=xt_t[b][0:1, 0:2, 0:128],
                start=True, stop=False, perf_mode=DR,
            )
            # block-diagonal scores: per q-tile j a [128k x 128q] tile
            for j in range(NJ):
                for half in range(2):
                    c = 2 * j + half
                    nc.tensor.matmul(
                        ph[
                            half * 64 : half * 64 + 64,
                            j * P + half * 64 : j * P + half * 64 + 64,
                        ],
                        lhsT=KTh[:, c * L : (c + 1) * L],
                        rhs=QTh[:, c * L : (c + 1) * L],
                        start=False,
                        stop=(half == 1 and j in (3, 4)),
                    )
            # term scores^T [128 terms, 640 q] in cols [640:1280]
            nc.tensor.matmul(
                ph[:, 640:1024], lhsT=KTh[:, NQ:S], rhs=QTh[:, 0:384],
                start=True, stop=True,
            )
            nc.tensor.matmul(
                ph[:, 1024:1280], lhsT=KTh[:, NQ:S], rhs=QTh[:, 384:640],
                start=True, stop=True,
            )
            # exp (ACT): terms + blocks in ONE instruction (both APs uniform)
            nc.scalar.activation(
                se[:, hh, :, :, :].rearrange("p j two q -> p two j q"),
                ph[:, 0:1280].rearrange("p (two j q) -> p two j q", j=NJ, q=P),
                AF.Exp, scale=0.125,
            )

        def vsums(b):
            """corr'[c] = sum of v over candidate tokens NOT in pair(c)."""
            vef = ve_t[b].rearrange("p k h c -> p k (h c)")
            psc = smt("pscor")
            for c0, nlen in ((0, 512), (512, 256)):
                cs = slice(c0, c0 + nlen)
                for kp in range(2):
                    nc.tensor.matmul(
                        psc[0:64, cs],
                        lhsT=notG6[:, 2 * kp : 2 * kp + 2, :],
                        rhs=vef[:, 2 * kp : 2 * kp + 2, cs],
                        start=(kp == 0),
                        stop=False,
                        perf_mode=DR,
                    )
                nc.tensor.matmul(
                    psc[0:CDD, cs],
                    lhsT=notG6[:, 4, 0:CDD],
                    rhs=vef[:, 4, cs],
                    start=False,
                    stop=True,
                )
            corrE = mp.tile([CDD, ZC + NH], FP8, tag="corr", name=f"corrE_{b}")
            ce_t[b] = corrE
            nc.vector.tensor_copy(corrE[:, 0:ZC], psc[0:CDD, 0:ZC])
            # Z constant: 0.25 * 512 masked-out keys, via indall (0/1 rows)
            nc.gpsimd.memset(corrE[:, ZC : ZC + NH], 128.0)

        def pv_j(b, j, tag, last=False):
            se, vext, corrE, stg = se_t[b], ve_t[b], ce_t[b], sg_t[b]
            psc = pjt("pspv") if tag == "pj" else smt("pspv")
            # corr opens both banks' accumulation groups
            nc.tensor.matmul(
                psc[:, 0:512], lhsT=indall[:, j, :], rhs=corrE[:, 0:512],
                start=True, stop=False,
            )
            nc.tensor.matmul(
                psc[:, 512:768], lhsT=indall[:, j, :], rhs=corrE[:, 512:768],
                start=True, stop=False,
            )
            nc.tensor.matmul(
                psc[:, ZC : ZC + NH], lhsT=indall[:, j, :],
                rhs=corrE[:, ZC : ZC + NH],
                start=False, stop=False,
            )
            for hh in range(NH):
                nc.tensor.matmul(
                    psc[:, hh * DH : (hh + 1) * DH],
                    lhsT=se[:, hh, j, :, :],
                    rhs=vext[:, j : KT : 5 - j, hh, :],
                    start=False,
                    stop=(hh == 7),
                    perf_mode=DR,
                )
                nc.tensor.matmul(
                    psc[:, ZC + hh : ZC + hh + 1],
                    lhsT=se[:, hh, j, :, :],
                    rhs=onecol2[:],
                    start=False,
                    stop=(hh == NH - 1),
                    perf_mode=DR,
                )
            zr = mp.tile([P, NH], FP32, tag="zr", bufs=2, name="zr")
            nc.vector.reciprocal(zr[:], psc[:, ZC : ZC + NH])
            halves = ((0, 6), (6, 12)) if last else ((0, 12),)
            for lo, hi in halves:
                in0 = psc[:, lo * DH : hi * DH].rearrange(
                    "p (h c) -> p h c", c=DH
                )
                in1 = zr[:, lo:hi].rearrange("p (h o) -> p h o", o=1)
                bin0, bin1 = bass.broadcast_tensor_aps(in0, in1)
                nc.vector.scalar_tensor_tensor(
                    out=stg[:, j, lo * DH : hi * DH].rearrange(
                        "p (h c) -> p h c", c=DH
                    ),
                    in0=bin0,
                    scalar=ONECOL,
                    in1=bin1,
                    op0=ALU.mult,
                    op1=ALU.mult,
                )
                nc.sync.dma_start(
                    out=out[b].rearrange("(r p) h -> p r h", p=P)[
                        :, j, lo * DH : hi * DH
                    ],
                    in_=stg[:, j, lo * DH : hi * DH],
                )

        # ---------------- schedule ----------------
        # Scores lag projections by one m-tile so each group's score matmuls
        # overlap the NEXT group's projections instead of waiting on their
        # own Q/K copies. Batch 0 PV hides inside batch 1's loop.
        alloc_bufs(0)
        alloc_bufs(1)

        for mt in range(KT):
            qk_mt(0, mt)
            if mt == 0:
                # bvb[p, o] = bv[o] broadcast (rank-1); after first group so
                # the PE isn't blocked on the bvrow DMA at t=0
                psb = smt("psbv")
                nc.tensor.matmul(psb[:, 0:512], lhsT=onesrow[:], rhs=bvrow[0:1, 0:512], start=True, stop=True)
                nc.tensor.matmul(psb[:, 512:768], lhsT=onesrow[:], rhs=bvrow[0:1, 512:768], start=True, stop=True)
                nc.vector.tensor_copy(bvb[:, 0:512], psb[:, 0:512])
                nc.vector.tensor_copy(bvb[:, 512:768], psb[:, 512:768])
            else:
                v_tt(0, mt - 1)
                scores_h(0, 2 * (mt - 1))
                scores_h(0, 2 * (mt - 1) + 1)
        v_tt(0, 4)
        v_term(0)
        vsums(0)
        scores_h(0, 10)
        scores_h(0, 11)

        for mt in range(KT):
            qk_mt(1, mt)
            if mt < 5:
                v_tt(1, mt)
            if mt >= 1:
                scores_h(1, 2 * (mt - 1))
                scores_h(1, 2 * (mt - 1) + 1)
                pv_j(0, mt - 1, "pv")
        v_term(1)
        vsums(1)
        scores_h(1, 10)
        scores_h(1, 11)

        # batch 1 PV, double-buffered via the now-idle 3-bank pj tag
        for j in range(NJ):
            pv_j(1, j, "pj" if j % 2 == 0 else "sm", last=(j == NJ - 1))


_CACHE = {}


def _get_program():
    if "nc" not in _CACHE:
        _CACHE["nc"] = _build_program()
    return _CACHE["nc"]


def _make_in_maps(inputs):
    f8 = ml_dtypes.float8_e4m3
    bf = ml_dtypes.bfloat16
    hs = np.asarray(inputs["hidden_states"], np.float32)
    hst = np.ascontiguousarray(hs.transpose(0, 2, 1))  # [B, H, S]
    wq = np.asarray(inputs["Wq"], np.float32).T * WS
    wk = np.asarray(inputs["Wk"], np.float32).T * WS
    wv = np.asarray(inputs["Wv"], np.float32).T
    in_common = {
        "wq8": np.ascontiguousarray(wq).astype(f8),
        "wk8": np.ascontiguousarray(wk).astype(f8),
        "wv8": np.ascontiguousarray(wv * WS).astype(f8),
        "wv16": np.ascontiguousarray(wv).astype(bf),
        "bq": np.asarray(inputs["bq"], np.float32),
        "bk": np.asarray(inputs["bk"], np.float32),
        "bv16": np.asarray(inputs["bv"], np.float32).astype(bf),
    }
    x8 = hst.astype(f8)
    # xt16 packed [BL, P, KT, T] so the DMA's contiguous runs are 1536B
    xm = (
        hst[:, :, NQ:]
        .reshape(B, KT, P, T)
        .transpose(0, 2, 1, 3)
        .astype(bf)
    )
    return [
        {
            "x8": x8[i * BL : (i + 1) * BL],
            "xt16": np.ascontiguousarray(xm[i * BL : (i + 1) * BL]),
            **in_common,
        }
        for i in range(NCORES)
    ]


def kernel(**inputs) -> np.ndarray:
    in_maps = _make_in_maps(inputs)
    nc = _get_program()
    res = run_bass_kernel_spmd(nc, in_maps, list(range(NCORES)))
    return np.concatenate([res.results[i]["out"] for i in range(NCORES)], axis=0)


# revision 4
# speedup vs baseline: 1.6982x; 1.0027x over previous
"""Trainium2 Bass kernel for nn_BertSelfAttention_79577154060613 (fp8).

Block-sparse BERT self-attention, data-parallel over batch across 8 cores
(2 batches/core). Cost-model-guided redesign of the bf16 baseline:

- All projections run as fp8e4m3 DoubleRow matmuls: 2 contraction k-tiles
  packed per instruction at 0.5 cycles/row -> 4x cheaper than bf16. Weights
  are pre-scaled by 16 on the host so w*16 lands in fp8's normal range; the
  PSUM->SBUF copies divide by 16 (free via the copy's scale port).
- The V projection of the 128 term tokens runs in bf16 (the reference output
  passes those rows through untouched, so they set the error floor). All
  other fp8 error sources only perturb softmax-averaged context and stay
  ~1e-3 absolute.
- Per head, term scores ([128 terms x 640 q]) and block scores land in one
  3-bank PSUM tile; block scores are computed as 5 block-diagonal [128x128]
  key-x-query tiles whose off-diagonal quadrants are zero-filled by two
  rank-1 DoubleRow matmuls. exp(0)=1 garbage in those quadrants contributes
  exactly sum(v over the sibling block), which the correction term absorbs
  by excluding the whole block PAIR instead of just the own block:
    ctx*Z = sum_{k in block|terms} e^s v_k + 1*Vsum_sibling + corr'(c)
    corr'(c) = sum_{c' not in pair(c)} Vsum_c'   (rank-10 matmul vs vext)
  This allows ONE exp instruction per head for all block scores.
- PV is a single DoubleRow matmul per (q-tile, head): contraction half A =
  128 exp'ed term scores, half B = the 128-row block-diagonal exp'ed block
  scores; rhs halves are vext[term-tile] and vext[j-tile]. The softmax
  denominator is accumulated by rank-1 DoubleRow matmuls (se @ 0.25-column)
  into columns [768:780] of the same PSUM pair, with the corr matmul
  contributing the 0.25*512 constant from masked-out keys; the heads' 64-col
  context slices are gapless so the divide is ONE DVE op per q-tile.
- Output is staged in SBUF [128, 6, 768] fp32 and DMA'd per 128-row slice
  (6 DMAs/batch). Phases are software-pipelined: batch0 PV hides inside
  batch1's projection loop; batch1 PV double-buffers via the then-idle
  3-bank PSUM tag.
"""

import numpy as np
import ml_dtypes

import concourse.bass as bass
import concourse.mybir as mybir
import concourse.tile as tile
from concourse import bacc
from concourse.bass_utils import run_bass_kernel_spmd

B, CDD, L, T, H, NH = 16, 10, 64, 128, 768, 12
DH = H // NH  # 64
S = CDD * L + T  # 768
NQ = CDD * L  # 640
P = 128
NCORES = 8
BL = B // NCORES  # 2
KT = H // P  # 6
FP32 = mybir.dt.float32
BF16 = mybir.dt.bfloat16
FP8 = mybir.dt.float8e4
AF = mybir.ActivationFunctionType
ALU = mybir.AluOpType
DR = mybir.MatmulPerfMode.DoubleRow
ONECOL = 0.25  # Z-column scale (keeps the corr Z constant fp8-exact: 128)
WS = 16.0  # host-side weight scale
IWS = 1.0 / WS
NJ = NQ // P  # 5 q-tiles
ZC = NH * DH  # 768: column where the Z region starts in psc


def _build_program():
    nc = bacc.Bacc(
        "TRN2", target_bir_lowering=False, debug=False, num_devices=NCORES
    )
    x8 = nc.dram_tensor("x8", [BL, H, S], FP8, kind="ExternalInput").ap()
    xt16 = nc.dram_tensor("xt16", [BL, P, KT, T], BF16, kind="ExternalInput").ap()
    wq8 = nc.dram_tensor("wq8", [H, H], FP8, kind="ExternalInput").ap()
    wk8 = nc.dram_tensor("wk8", [H, H], FP8, kind="ExternalInput").ap()
    wv8 = nc.dram_tensor("wv8", [H, H], FP8, kind="ExternalInput").ap()
    wv16 = nc.dram_tensor("wv16", [H, H], BF16, kind="ExternalInput").ap()
    bq = nc.dram_tensor("bq", [H], FP32, kind="ExternalInput").ap()
    bk = nc.dram_tensor("bk", [H], FP32, kind="ExternalInput").ap()
    bv16 = nc.dram_tensor("bv16", [H], BF16, kind="ExternalInput").ap()
    out = nc.dram_tensor("out", [BL, S, H], FP32, kind="ExternalOutput").ap()

    with tile.TileContext(nc) as tc:
        _emit(tc, nc, x8, xt16, wq8, wk8, wv8, wv16, bq, bk, bv16, out)
    nc.compile()
    return nc


def _emit(tc, nc, x8, xt16, wq8, wk8, wv8, wv16, bq, bk, bv16, out):
    from contextlib import ExitStack

    ctx = ExitStack()
    with ctx:
        cp = ctx.enter_context(tc.tile_pool(name="consts", bufs=1))
        wp = ctx.enter_context(tc.tile_pool(name="weights", bufs=1))
        xp = ctx.enter_context(tc.tile_pool(name="xin", bufs=2))
        qp = ctx.enter_context(tc.tile_pool(name="qkv", bufs=2))
        sp = ctx.enter_context(tc.tile_pool(name="sexp", bufs=2))
        op = ctx.enter_context(tc.tile_pool(name="ostg", bufs=2))
        mp = ctx.enter_context(tc.tile_pool(name="small", bufs=2))
        pa = ctx.enter_context(tc.tile_pool(name="ps", bufs=1, space="PSUM"))

        def pjt(name):
            return pa.tile(
                [P, 1536], FP32, tag="pj", bufs=2, name=name,
                padded_shape=[P, 1536],
            )

        def smt(name):
            return pa.tile(
                [P, 1024], FP32, tag="sm", bufs=1, name=name,
                padded_shape=[P, 1024],
            )

        # ---------------- input DMAs (critical-path order) ----------------
        # first Q m-tile needs wq cols [0:128] and xt kt-pair 0 only: split
        # those DMAs so the PE can start ~1.5us in
        wq_sb = wp.tile([P, KT, H], FP8, name="wq8sb")
        wq_r = wq8.rearrange("(k p) o -> p k o", p=P)
        nc.sync.dma_start(out=wq_sb[:, :, 0:P], in_=wq_r[:, :, 0:P])
        xt_t, xm_t = [], []
        for b in range(BL):
            xt_t.append(xp.tile([P, KT, S], FP8, tag="xt", name=f"xt{b}"))
            xm_t.append(xp.tile([P, KT, T], BF16, tag="xm", name=f"xm{b}"))
        x0_r = x8[0].rearrange("(k p) s -> p k s", p=P)
        nc.sync.dma_start(out=xt_t[0][:, 0:2, :], in_=x0_r[:, 0:2, :])
        bcq = cp.tile([P, KT], FP32, name="bcq")
        nc.sync.dma_start(out=bcq[:], in_=bq.rearrange("(t p) -> p t", p=P))
        bck = cp.tile([P, KT], FP32, name="bck")
        nc.sync.dma_start(out=bck[:], in_=bk.rearrange("(t p) -> p t", p=P))
        nc.sync.dma_start(out=xt_t[0][:, 2:4, :], in_=x0_r[:, 2:4, :])
        nc.sync.dma_start(out=xt_t[0][:, 4:6, :], in_=x0_r[:, 4:6, :])
        wk_sb = wp.tile([P, KT, H], FP8, name="wk8sb")
        wk_r = wk8.rearrange("(k p) o -> p k o", p=P)
        nc.sync.dma_start(out=wk_sb[:, :, 0:P], in_=wk_r[:, :, 0:P])
        nc.sync.dma_start(out=wq_sb[:, :, P:H], in_=wq_r[:, :, P:H])
        nc.sync.dma_start(out=wk_sb[:, :, P:H], in_=wk_r[:, :, P:H])
        wv_sb = wp.tile([P, KT, H], FP8, name="wv8sb")
        nc.sync.dma_start(out=wv_sb[:], in_=wv8.rearrange("(k p) o -> p k o", p=P))
        bvrow = cp.tile([1, H], BF16, name="bvrow")
        nc.sync.dma_start(out=bvrow[:], in_=bv16[None, :])
        nc.sync.dma_start(out=xt_t[1][:], in_=x8[1].rearrange("(k p) s -> p k s", p=P))
        wv16_sb = wp.tile([P, KT, H], BF16, name="wv16sb")
        nc.sync.dma_start(out=wv16_sb[:], in_=wv16.rearrange("(k p) o -> p k o", p=P))
        nc.sync.dma_start(out=xm_t[0][:], in_=xt16[0])
        nc.sync.dma_start(out=xm_t[1][:], in_=xt16[1])

        # ---------------- constants (Pool) ----------------
        onesrow = cp.tile([1, P], FP8, name="onesrow")
        nc.gpsimd.memset(onesrow[:], 1.0)
        zpair = cp.tile([1, 2, P], FP8, name="zpair")
        nc.gpsimd.memset(zpair[:], 0.0)
        onecol2 = cp.tile([P, 2, 1], FP8, name="onecol2")
        nc.gpsimd.memset(onecol2[:], ONECOL)
        # notG6[p, kt, c] = 0 if block c is in tile kt's pair else 1.
        # Inner dim padded to 64 so dual-fp8 LdWeights half-stride is aligned
        # (cols 10:64 are zero -> psum rows 10:64 unused).
        notG6 = cp.tile([P, KT, 64], FP8, name="notG6")
        nc.gpsimd.memset(notG6[:], 0.0)
        nc.gpsimd.memset(notG6[:, :, 0:CDD], 1.0)
        for kt in range(5):
            nc.gpsimd.memset(notG6[:, kt, 2 * kt : 2 * kt + 2], 0.0)
        # indall[c, j, q] = 1 iff query q of tile j belongs to block c,
        # i.e. c - 2j - (q // 64) == 0
        indall = cp.tile([CDD, NJ, P], FP8, name="indall")
        nc.gpsimd.memset(indall[:], 1.0)
        nc.gpsimd.affine_select(
            out=indall.rearrange("c j (h q) -> c j h q", q=64),
            in_=indall.rearrange("c j (h q) -> c j h q", q=64),
            compare_op=ALU.is_equal,
            fill=0.0,
            base=0,
            pattern=[[-2, NJ], [-1, 2], [0, 64]],
            channel_multiplier=1,
        )
        bvb = cp.tile([P, H], FP32, name="bvb")

        qt_t = [None] * BL
        kt_t = [None] * BL
        ve_t = [None] * BL
        se_t = [None] * BL
        ce_t = [None] * BL
        sg_t = [None] * BL

        wq8_z = None  # zero-fill rhs uses xt slices (DMA'd first)

        def alloc_bufs(b):
            qt_t[b] = qp.tile([P, KT, NQ], FP8, tag="qt", name=f"qt8_{b}")
            kt_t[b] = qp.tile([P, KT, S], FP8, tag="kt", name=f"kt8_{b}")
            ve_t[b] = qp.tile([P, KT, NH, DH], FP8, tag="ve", name=f"vext_{b}")
            se_t[b] = sp.tile([P, NH, NJ, 2, P], FP8, tag="se", name=f"se_{b}")
            sg_t[b] = op.tile([P, KT, H], FP32, tag="stg", name=f"stg_{b}")

        def qk_mt(b, mt):
            """Q and K projections for one m-tile, sharing a 3-bank psum:
            Q at [0:512]+[512:640], K at [640:1024]+[1024:1408]."""
            ms = slice(mt * P, (mt + 1) * P)
            ps = pjt("psqk")
            for c0, nlen in ((0, 512), (512, 128)):
                for kp in range(3):
                    nc.tensor.matmul(
                        ps[:, c0 : c0 + nlen],
                        lhsT=wq_sb[:, 2 * kp : 2 * kp + 2, ms],
                        rhs=xt_t[b][:, 2 * kp : 2 * kp + 2, c0 : c0 + nlen],
                        start=(kp == 0),
                        stop=(kp == 2),
                        perf_mode=DR,
                    )
            for c0, nlen in ((640, 384), (1024, 384)):
                for kp in range(3):
                    nc.tensor.matmul(
                        ps[:, c0 : c0 + nlen],
                        lhsT=wk_sb[:, 2 * kp : 2 * kp + 2, ms],
                        rhs=xt_t[b][:, 2 * kp : 2 * kp + 2, c0 - 640 : c0 - 640 + nlen],
                        start=(kp == 0),
                        stop=(kp == 2),
                        perf_mode=DR,
                    )
            if mt < 5 or b == 1:  # balance: most Q copies on ACT
                nc.scalar.activation(
                    qt_t[b][:, mt, :], ps[:, 0:NQ], AF.Identity,
                    bias=bcq[:, mt : mt + 1], scale=IWS,
                )
            else:
                nc.vector.tensor_scalar(
                    qt_t[b][:, mt, :], ps[:, 0:NQ], IWS, bcq[:, mt : mt + 1],
                    op0=ALU.mult, op1=ALU.add,
                )
            nc.vector.tensor_scalar(
                kt_t[b][:, mt, :], ps[:, 640 : 640 + S], IWS, bck[:, mt : mt + 1],
                op0=ALU.mult, op1=ALU.add,
            )

        def v_tt(b, tt):
            """Candidate-token V tile (fp8)."""
            ts = slice(tt * P, (tt + 1) * P)
            psv = smt("psv")
            for c0, nlen in ((0, 512), (512, 256)):
                for kp in range(3):
                    nc.tensor.matmul(
                        psv[:, c0 : c0 + nlen],
                        lhsT=xt_t[b][:, 2 * kp : 2 * kp + 2, ts],
                        rhs=wv_sb[:, 2 * kp : 2 * kp + 2, c0 : c0 + nlen],
                        start=(kp == 0),
                        stop=(kp == 2),
                        perf_mode=DR,
                    )
            nc.vector.scalar_tensor_tensor(
                out=ve_t[b][:, tt, :, :],
                in0=psv[:, 0:H].rearrange("p (h c) -> p h c", c=DH),
                scalar=IWS,
                in1=bvb.rearrange("p (h c) -> p h c", c=DH),
                op0=ALU.mult,
                op1=ALU.add,
            )

        def v_term(b):
            """Term-token V in bf16 (output passthrough accuracy)."""
            psv = smt("psvt")
            for c0, nlen in ((0, 512), (512, 256)):
                for kt in range(KT):
                    nc.tensor.matmul(
                        psv[:, c0 : c0 + nlen],
                        lhsT=xm_t[b][:, kt, :],
                        rhs=wv16_sb[:, kt, c0 : c0 + nlen],
                        start=(kt == 0),
                        stop=(kt == KT - 1),
                    )
            stg = sg_t[b]
            nc.vector.tensor_tensor(
                out=stg[:, 5, :], in0=psv[:, 0:H], in1=bvb[:], op=ALU.add
            )
            nc.gpsimd.tensor_copy(
                ve_t[b][:, 5, :, :],
                stg[:, 5, :].rearrange("p (h c) -> p h c", c=DH),
            )
            nc.sync.dma_start(
                out=out[b].rearrange("(r p) h -> p r h", p=P)[:, 5, :],
                in_=stg[:, 5, :],
            )

        def scores_h(b, hh):
            mt, hl = hh // 2, hh % 2
            r0 = hl * 64
            KTh = kt_t[b][r0 : r0 + 64, mt, :]
            QTh = qt_t[b][r0 : r0 + 64, mt, :]
            se = se_t[b]
            ph = pjt("ph")
            # se half 0 = block scores (pairs with vext[j] in the PV DR
            # matmul), half 1 = term scores (pairs with vext[5]).
            # zero-fill block region [0:640] (banks 0-1)
            nc.tensor.matmul(
                ph[:, 0:512], lhsT=zpair[:], rhs=xt_t[b][0:1, 0:2, 0:512],
                start=True, stop=False, perf_mode=DR,
            )
            nc.tensor.matmul(
                ph[:, 512:640], lhsT=zpair[:], rhs=xt_t[b][0:1, 0:2, 0:128],
                start=True, stop=False, perf_mode=DR,
            )
            # block-diagonal scores: per q-tile j a [128k x 128q] tile
            for j in range(NJ):
                for half in range(2):
                    c = 2 * j + half
                    nc.tensor.matmul(
                        ph[
                            half * 64 : half * 64 + 64,
                            j * P + half * 64 : j * P + half * 64 + 64,
                        ],
                        lhsT=KTh[:, c * L : (c + 1) * L],
                        rhs=QTh[:, c * L : (c + 1) * L],
                        start=False,
                        stop=(half == 1 and j in (3, 4)),
                    )
            # term scores^T [128 terms, 640 q] in cols [640:1280]
            nc.tensor.matmul(
                ph[:, 640:1024], lhsT=KTh[:, NQ:S], rhs=QTh[:, 0:384],
                start=True, stop=True,
            )
            nc.tensor.matmul(
                ph[:, 1024:1280], lhsT=KTh[:, NQ:S], rhs=QTh[:, 384:640],
                start=True, stop=True,
            )
            # exp (ACT): terms + blocks in ONE instruction (both APs uniform)
            nc.scalar.activation(
                se[:, hh, :, :, :].rearrange("p j two q -> p two j q"),
                ph[:, 0:1280].rearrange("p (two j q) -> p two j q", j=NJ, q=P),
                AF.Exp, scale=0.125,
            )

        def vsums(b):
            """corr'[c] = sum of v over candidate tokens NOT in pair(c)."""
            vef = ve_t[b].rearrange("p k h c -> p k (h c)")
            psc = smt("pscor")
            for c0, nlen in ((0, 512), (512, 256)):
                cs = slice(c0, c0 + nlen)
                for kp in range(2):
                    nc.tensor.matmul(
                        psc[0:64, cs],
                        lhsT=notG6[:, 2 * kp : 2 * kp + 2, :],
                        rhs=vef[:, 2 * kp : 2 * kp + 2, cs],
                        start=(kp == 0),
                        stop=False,
                        perf_mode=DR,
                    )
                nc.tensor.matmul(
                    psc[0:CDD, cs],
                    lhsT=notG6[:, 4, 0:CDD],
                    rhs=vef[:, 4, cs],
                    start=False,
                    stop=True,
                )
            corrE = mp.tile([CDD, ZC + NH], FP8, tag="corr", name=f"corrE_{b}")
            ce_t[b] = corrE
            nc.vector.tensor_copy(corrE[:, 0:ZC], psc[0:CDD, 0:ZC])
            # Z constant: 0.25 * 512 masked-out keys, via indall (0/1 rows)
            nc.gpsimd.memset(corrE[:, ZC : ZC + NH], 128.0)

        def pv_j(b, j, tag, last=False):
            se, vext, corrE, stg = se_t[b], ve_t[b], ce_t[b], sg_t[b]
            psc = pjt("pspv") if tag == "pj" else smt("pspv")
            # corr opens both banks' accumulation groups
            nc.tensor.matmul(
                psc[:, 0:512], lhsT=indall[:, j, :], rhs=corrE[:, 0:512],
                start=True, stop=False,
            )
            nc.tensor.matmul(
                psc[:, 512:768], lhsT=indall[:, j, :], rhs=corrE[:, 512:768],
                start=True, stop=False,
            )
            nc.tensor.matmul(
                psc[:, ZC : ZC + NH], lhsT=indall[:, j, :],
                rhs=corrE[:, ZC : ZC + NH],
                start=False, stop=False,
            )
            for hh in range(NH):
                nc.tensor.matmul(
                    psc[:, hh * DH : (hh + 1) * DH],
                    lhsT=se[:, hh, j, :, :],
                    rhs=vext[:, j : KT : 5 - j, hh, :],
                    start=False,
                    stop=(hh == 7),
                    perf_mode=DR,
                )
                nc.tensor.matmul(
                    psc[:, ZC + hh : ZC + hh + 1],
                    lhsT=se[:, hh, j, :, :],
                    rhs=onecol2[:],
                    start=False,
                    stop=(hh == NH - 1),
                    perf_mode=DR,
                )
            zr = mp.tile([P, NH], FP32, tag="zr", bufs=2, name="zr")
            nc.vector.reciprocal(zr[:], psc[:, ZC : ZC + NH])
            halves = ((0, 6), (6, 12)) if last else ((0, 12),)
            for lo, hi in halves:
                in0 = psc[:, lo * DH : hi * DH].rearrange(
                    "p (h c) -> p h c", c=DH
                )
                in1 = zr[:, lo:hi].rearrange("p (h o) -> p h o", o=1)
                bin0, bin1 = bass.broadcast_tensor_aps(in0, in1)
                nc.vector.scalar_tensor_tensor(
                    out=stg[:, j, lo * DH : hi * DH].rearrange(
                        "p (h c) -> p h c", c=DH
                    ),
                    in0=bin0,
                    scalar=ONECOL,
                    in1=bin1,
                    op0=ALU.mult,
                    op1=ALU.mult,
                )
                nc.sync.dma_start(
                    out=out[b].rearrange("(r p) h -> p r h", p=P)[
                        :, j, lo * DH : hi * DH
                    ],
                    in_=stg[:, j, lo * DH : hi * DH],
                )

        # ---------------- schedule ----------------
        # Scores lag projections by one m-tile so each group's score matmuls
        # overlap the NEXT group's projections instead of waiting on their
        # own Q/K copies. Batch 0 PV hides inside batch 1's loop.
        alloc_bufs(0)
        alloc_bufs(1)

        for mt in range(KT):
            qk_mt(0, mt)
            if mt == 0:
                # bvb[p, o] = bv[o] broadcast (rank-1); after first group so
                # the PE isn't blocked on the bvrow DMA at t=0
                psb = smt("psbv")
                nc.tensor.matmul(psb[:, 0:512], lhsT=onesrow[:], rhs=bvrow[0:1, 0:512], start=True, stop=True)
                nc.tensor.matmul(psb[:, 512:768], lhsT=onesrow[:], rhs=bvrow[0:1, 512:768], start=True, stop=True)
                nc.vector.tensor_copy(bvb[:, 0:512], psb[:, 0:512])
                nc.vector.tensor_copy(bvb[:, 512:768], psb[:, 512:768])
            else:
                v_tt(0, mt - 1)
                scores_h(0, 2 * (mt - 1))
                scores_h(0, 2 * (mt - 1) + 1)
        v_tt(0, 4)
        v_term(0)
        vsums(0)
        scores_h(0, 10)
        scores_h(0, 11)

        for mt in range(KT):
            qk_mt(1, mt)
            if mt < 5:
                v_tt(1, mt)
            if mt >= 1:
                scores_h(1, 2 * (mt - 1))
                scores_h(1, 2 * (mt - 1) + 1)
            if mt >= 2:
                pv_j(0, mt - 2, "pv")
        v_term(1)
        vsums(1)
        scores_h(1, 10)
        scores_h(1, 11)
        pv_j(0, 4, "pv")

        # batch 1 PV, double-buffered via the now-idle 3-bank pj tag
        for j in range(NJ):
            pv_j(1, j, "pj", last=(j == NJ - 1))


_CACHE = {}


def _get_program():
    if "nc" not in _CACHE:
        _CACHE["nc"] = _build_program()
    return _CACHE["nc"]


def _make_in_maps(inputs):
    f8 = ml_dtypes.float8_e4m3
    bf = ml_dtypes.bfloat16
    hs = np.asarray(inputs["hidden_states"], np.float32)
    hst = np.ascontiguousarray(hs.transpose(0, 2, 1))  # [B, H, S]
    wq = np.asarray(inputs["Wq"], np.float32).T * WS
    wk = np.asarray(inputs["Wk"], np.float32).T * WS
    wv = np.asarray(inputs["Wv"], np.float32).T
    in_common = {
        "wq8": np.ascontiguousarray(wq).astype(f8),
        "wk8": np.ascontiguousarray(wk).astype(f8),
        "wv8": np.ascontiguousarray(wv * WS).astype(f8),
        "wv16": np.ascontiguousarray(wv).astype(bf),
        "bq": np.asarray(inputs["bq"], np.float32),
        "bk": np.asarray(inputs["bk"], np.float32),
        "bv16": np.asarray(inputs["bv"], np.float32).astype(bf),
    }
    x8 = hst.astype(f8)
    # xt16 packed [BL, P, KT, T] so the DMA's contiguous runs are 1536B
    xm = (
        hst[:, :, NQ:]
        .reshape(B, KT, P, T)
        .transpose(0, 2, 1, 3)
        .astype(bf)
    )
    return [
        {
            "x8": x8[i * BL : (i + 1) * BL],
            "xt16": np.ascontiguousarray(xm[i * BL : (i + 1) * BL]),
            **in_common,
        }
        for i in range(NCORES)
    ]


def kernel(**inputs) -> np.ndarray:
    in_maps = _make_in_maps(inputs)
    nc = _get_program()
    res = run_bass_kernel_spmd(nc, in_maps, list(range(NCORES)))
    return np.concatenate([res.results[i]["out"] for i in range(NCORES)], axis=0)


# revision 5
# speedup vs baseline: 1.7010x; 1.0017x over previous
"""Trainium2 Bass kernel for nn_BertSelfAttention_79577154060613 (fp8).

Block-sparse BERT self-attention, data-parallel over batch across 8 cores
(2 batches/core). Cost-model-guided redesign of the bf16 baseline:

- All projections run as fp8e4m3 DoubleRow matmuls: 2 contraction k-tiles
  packed per instruction at 0.5 cycles/row -> 4x cheaper than bf16. Weights
  are pre-scaled by 16 on the host so w*16 lands in fp8's normal range; the
  PSUM->SBUF copies divide by 16 (free via the copy's scale port).
- The V projection of the 128 term tokens runs in bf16 (the reference output
  passes those rows through untouched, so they set the error floor). All
  other fp8 error sources only perturb softmax-averaged context and stay
  ~1e-3 absolute.
- Per head, term scores ([128 terms x 640 q]) and block scores land in one
  3-bank PSUM tile; block scores are computed as 5 block-diagonal [128x128]
  key-x-query tiles whose off-diagonal quadrants are zero-filled by two
  rank-1 DoubleRow matmuls. exp(0)=1 garbage in those quadrants contributes
  exactly sum(v over the sibling block), which the correction term absorbs
  by excluding the whole block PAIR instead of just the own block:
    ctx*Z = sum_{k in block|terms} e^s v_k + 1*Vsum_sibling + corr'(c)
    corr'(c) = sum_{c' not in pair(c)} Vsum_c'   (rank-10 matmul vs vext)
  This allows ONE exp instruction per head for all block scores.
- PV is a single DoubleRow matmul per (q-tile, head): contraction half A =
  128 exp'ed term scores, half B = the 128-row block-diagonal exp'ed block
  scores; rhs halves are vext[term-tile] and vext[j-tile]. The softmax
  denominator is accumulated by rank-1 DoubleRow matmuls (se @ 0.25-column)
  into columns [768:780] of the same PSUM pair, with the corr matmul
  contributing the 0.25*512 constant from masked-out keys; the heads' 64-col
  context slices are gapless so the divide is ONE DVE op per q-tile.
- Output is staged in SBUF [128, 6, 768] fp32 and DMA'd per 128-row slice
  (6 DMAs/batch). Phases are software-pipelined: batch0 PV hides inside
  batch1's projection loop; batch1 PV double-buffers via the then-idle
  3-bank PSUM tag.
"""

import numpy as np
import ml_dtypes

import concourse.bass as bass
import concourse.mybir as mybir
import concourse.tile as tile
from concourse import bacc
from concourse.bass_utils import run_bass_kernel_spmd

B, CDD, L, T, H, NH = 16, 10, 64, 128, 768, 12
DH = H // NH  # 64
S = CDD * L + T  # 768
NQ = CDD * L  # 640
P = 128
NCORES = 8
BL = B // NCORES  # 2
KT = H // P  # 6
FP32 = mybir.dt.float32
BF16 = mybir.dt.bfloat16
FP8 = mybir.dt.float8e4
AF = mybir.ActivationFunctionType
ALU = mybir.AluOpType
DR = mybir.MatmulPerfMode.DoubleRow
ONECOL = 0.25  # Z-column scale (keeps the corr Z constant fp8-exact: 128)
WS = 16.0  # host-side weight scale
IWS = 1.0 / WS
NJ = NQ // P  # 5 q-tiles
ZC = NH * DH  # 768: column where the Z region starts in psc


def _build_program():
    nc = bacc.Bacc(
        "TRN2", target_bir_lowering=False, debug=False, num_devices=NCORES
    )
    x8 = nc.dram_tensor("x8", [BL, H, S], FP8, kind="ExternalInput").ap()
    xt16 = nc.dram_tensor("xt16", [BL, P, KT, T], BF16, kind="ExternalInput").ap()
    wq8 = nc.dram_tensor("wq8", [H, H], FP8, kind="ExternalInput").ap()
    wk8 = nc.dram_tensor("wk8", [H, H], FP8, kind="ExternalInput").ap()
    wv8 = nc.dram_tensor("wv8", [H, H], FP8, kind="ExternalInput").ap()
    wv16 = nc.dram_tensor("wv16", [H, H], BF16, kind="ExternalInput").ap()
    bq = nc.dram_tensor("bq", [H], FP32, kind="ExternalInput").ap()
    bk = nc.dram_tensor("bk", [H], FP32, kind="ExternalInput").ap()
    bv16 = nc.dram_tensor("bv16", [H], BF16, kind="ExternalInput").ap()
    out = nc.dram_tensor("out", [BL, S, H], FP32, kind="ExternalOutput").ap()

    with tile.TileContext(nc) as tc:
        _emit(tc, nc, x8, xt16, wq8, wk8, wv8, wv16, bq, bk, bv16, out)
    nc.compile()
    return nc


def _emit(tc, nc, x8, xt16, wq8, wk8, wv8, wv16, bq, bk, bv16, out):
    from contextlib import ExitStack

    ctx = ExitStack()
    with ctx:
        cp = ctx.enter_context(tc.tile_pool(name="consts", bufs=1))
        wp = ctx.enter_context(tc.tile_pool(name="weights", bufs=1))
        xp = ctx.enter_context(tc.tile_pool(name="xin", bufs=2))
        qp = ctx.enter_context(tc.tile_pool(name="qkv", bufs=2))
        sp = ctx.enter_context(tc.tile_pool(name="sexp", bufs=2))
        op = ctx.enter_context(tc.tile_pool(name="ostg", bufs=2))
        mp = ctx.enter_context(tc.tile_pool(name="small", bufs=2))
        pa = ctx.enter_context(tc.tile_pool(name="ps", bufs=1, space="PSUM"))

        def pjt(name):
            return pa.tile(
                [P, 1536], FP32, tag="pj", bufs=2, name=name,
                padded_shape=[P, 1536],
            )

        def smt(name):
            return pa.tile(
                [P, 1024], FP32, tag="sm", bufs=1, name=name,
                padded_shape=[P, 1024],
            )

        # ---------------- input DMAs (critical-path order) ----------------
        # first Q m-tile needs wq cols [0:128] and xt kt-pair 0 only: split
        # those DMAs so the PE can start ~1.5us in
        wq_sb = wp.tile([P, KT, H], FP8, name="wq8sb")
        wq_r = wq8.rearrange("(k p) o -> p k o", p=P)
        nc.sync.dma_start(out=wq_sb[:, :, 0:P], in_=wq_r[:, :, 0:P])
        xt_t, xm_t = [], []
        for b in range(BL):
            xt_t.append(xp.tile([P, KT, S], FP8, tag="xt", name=f"xt{b}"))
            xm_t.append(xp.tile([P, KT, T], BF16, tag="xm", name=f"xm{b}"))
        x0_r = x8[0].rearrange("(k p) s -> p k s", p=P)
        nc.sync.dma_start(out=xt_t[0][:, 0:2, :], in_=x0_r[:, 0:2, :])
        nc.sync.dma_start(out=xt_t[0][:, 2:4, :], in_=x0_r[:, 2:4, :])
        nc.sync.dma_start(out=xt_t[0][:, 4:6, :], in_=x0_r[:, 4:6, :])
        wk_sb = wp.tile([P, KT, H], FP8, name="wk8sb")
        wk_r = wk8.rearrange("(k p) o -> p k o", p=P)
        nc.sync.dma_start(out=wk_sb[:, :, 0:P], in_=wk_r[:, :, 0:P])
        bcq = cp.tile([P, KT], FP32, name="bcq")
        nc.sync.dma_start(out=bcq[:], in_=bq.rearrange("(t p) -> p t", p=P))
        bck = cp.tile([P, KT], FP32, name="bck")
        nc.sync.dma_start(out=bck[:], in_=bk.rearrange("(t p) -> p t", p=P))
        nc.sync.dma_start(out=wq_sb[:, :, P:H], in_=wq_r[:, :, P:H])
        nc.sync.dma_start(out=wk_sb[:, :, P:H], in_=wk_r[:, :, P:H])
        wv_sb = wp.tile([P, KT, H], FP8, name="wv8sb")
        nc.sync.dma_start(out=wv_sb[:], in_=wv8.rearrange("(k p) o -> p k o", p=P))
        bvrow = cp.tile([1, H], BF16, name="bvrow")
        nc.sync.dma_start(out=bvrow[:], in_=bv16[None, :])
        nc.sync.dma_start(out=xt_t[1][:], in_=x8[1].rearrange("(k p) s -> p k s", p=P))
        wv16_sb = wp.tile([P, KT, H], BF16, name="wv16sb")
        nc.sync.dma_start(out=wv16_sb[:], in_=wv16.rearrange("(k p) o -> p k o", p=P))
        nc.sync.dma_start(out=xm_t[0][:], in_=xt16[0])
        nc.sync.dma_start(out=xm_t[1][:], in_=xt16[1])

        # ---------------- constants (Pool) ----------------
        onesrow = cp.tile([1, P], FP8, name="onesrow")
        nc.gpsimd.memset(onesrow[:], 1.0)
        zpair = cp.tile([1, 2, P], FP8, name="zpair")
        nc.gpsimd.memset(zpair[:], 0.0)
        onecol2 = cp.tile([P, 2, 1], FP8, name="onecol2")
        nc.gpsimd.memset(onecol2[:], ONECOL)
        # notG6[p, kt, c] = 0 if block c is in tile kt's pair else 1.
        # Inner dim padded to 64 so dual-fp8 LdWeights half-stride is aligned
        # (cols 10:64 are zero -> psum rows 10:64 unused).
        notG6 = cp.tile([P, KT, 64], FP8, name="notG6")
        nc.gpsimd.memset(notG6[:], 0.0)
        nc.gpsimd.memset(notG6[:, :, 0:CDD], 1.0)
        for kt in range(5):
            nc.gpsimd.memset(notG6[:, kt, 2 * kt : 2 * kt + 2], 0.0)
        # indall[c, j, q] = 1 iff query q of tile j belongs to block c,
        # i.e. c - 2j - (q // 64) == 0
        indall = cp.tile([CDD, NJ, P], FP8, name="indall")
        nc.gpsimd.memset(indall[:], 1.0)
        nc.gpsimd.affine_select(
            out=indall.rearrange("c j (h q) -> c j h q", q=64),
            in_=indall.rearrange("c j (h q) -> c j h q", q=64),
            compare_op=ALU.is_equal,
            fill=0.0,
            base=0,
            pattern=[[-2, NJ], [-1, 2], [0, 64]],
            channel_multiplier=1,
        )
        bvb = cp.tile([P, H], FP32, name="bvb")

        qt_t = [None] * BL
        kt_t = [None] * BL
        ve_t = [None] * BL
        se_t = [None] * BL
        ce_t = [None] * BL
        sg_t = [None] * BL

        wq8_z = None  # zero-fill rhs uses xt slices (DMA'd first)

        def alloc_bufs(b):
            qt_t[b] = qp.tile([P, KT, NQ], FP8, tag="qt", name=f"qt8_{b}")
            kt_t[b] = qp.tile([P, KT, S], FP8, tag="kt", name=f"kt8_{b}")
            ve_t[b] = qp.tile([P, KT, NH, DH], FP8, tag="ve", name=f"vext_{b}")
            se_t[b] = sp.tile([P, NH, NJ, 2, P], FP8, tag="se", name=f"se_{b}")
            sg_t[b] = op.tile([P, KT, H], FP32, tag="stg", name=f"stg_{b}")

        def qk_mt(b, mt):
            """Q and K projections for one m-tile, sharing a 3-bank psum:
            Q at [0:512]+[512:640], K at [640:1024]+[1024:1408]."""
            ms = slice(mt * P, (mt + 1) * P)
            ps = pjt("psqk")
            for c0, nlen in ((0, 512), (512, 128)):
                for kp in range(3):
                    nc.tensor.matmul(
                        ps[:, c0 : c0 + nlen],
                        lhsT=wq_sb[:, 2 * kp : 2 * kp + 2, ms],
                        rhs=xt_t[b][:, 2 * kp : 2 * kp + 2, c0 : c0 + nlen],
                        start=(kp == 0),
                        stop=(kp == 2),
                        perf_mode=DR,
                    )
            for c0, nlen in ((640, 384), (1024, 384)):
                for kp in range(3):
                    nc.tensor.matmul(
                        ps[:, c0 : c0 + nlen],
                        lhsT=wk_sb[:, 2 * kp : 2 * kp + 2, ms],
                        rhs=xt_t[b][:, 2 * kp : 2 * kp + 2, c0 - 640 : c0 - 640 + nlen],
                        start=(kp == 0),
                        stop=(kp == 2),
                        perf_mode=DR,
                    )
            if mt < 5 or b == 1:  # balance: most Q copies on ACT
                nc.scalar.activation(
                    qt_t[b][:, mt, :], ps[:, 0:NQ], AF.Identity,
                    bias=bcq[:, mt : mt + 1], scale=IWS,
                )
            else:
                nc.vector.tensor_scalar(
                    qt_t[b][:, mt, :], ps[:, 0:NQ], IWS, bcq[:, mt : mt + 1],
                    op0=ALU.mult, op1=ALU.add,
                )
            nc.vector.tensor_scalar(
                kt_t[b][:, mt, :], ps[:, 640 : 640 + S], IWS, bck[:, mt : mt + 1],
                op0=ALU.mult, op1=ALU.add,
            )

        def v_tt(b, tt):
            """Candidate-token V tile (fp8)."""
            ts = slice(tt * P, (tt + 1) * P)
            psv = smt("psv")
            for c0, nlen in ((0, 512), (512, 256)):
                for kp in range(3):
                    nc.tensor.matmul(
                        psv[:, c0 : c0 + nlen],
                        lhsT=xt_t[b][:, 2 * kp : 2 * kp + 2, ts],
                        rhs=wv_sb[:, 2 * kp : 2 * kp + 2, c0 : c0 + nlen],
                        start=(kp == 0),
                        stop=(kp == 2),
                        perf_mode=DR,
                    )
            nc.vector.scalar_tensor_tensor(
                out=ve_t[b][:, tt, :, :],
                in0=psv[:, 0:H].rearrange("p (h c) -> p h c", c=DH),
                scalar=IWS,
                in1=bvb.rearrange("p (h c) -> p h c", c=DH),
                op0=ALU.mult,
                op1=ALU.add,
            )

        def v_term(b):
            """Term-token V in bf16 (output passthrough accuracy)."""
            psv = smt("psvt")
            for c0, nlen in ((0, 512), (512, 256)):
                for kt in range(KT):
                    nc.tensor.matmul(
                        psv[:, c0 : c0 + nlen],
                        lhsT=xm_t[b][:, kt, :],
                        rhs=wv16_sb[:, kt, c0 : c0 + nlen],
                        start=(kt == 0),
                        stop=(kt == KT - 1),
                    )
            stg = sg_t[b]
            nc.vector.tensor_tensor(
                out=stg[:, 5, :], in0=psv[:, 0:H], in1=bvb[:], op=ALU.add
            )
            nc.gpsimd.tensor_copy(
                ve_t[b][:, 5, :, :],
                stg[:, 5, :].rearrange("p (h c) -> p h c", c=DH),
            )
            nc.sync.dma_start(
                out=out[b].rearrange("(r p) h -> p r h", p=P)[:, 5, :],
                in_=stg[:, 5, :],
            )

        def scores_h(b, hh):
            mt, hl = hh // 2, hh % 2
            r0 = hl * 64
            KTh = kt_t[b][r0 : r0 + 64, mt, :]
            QTh = qt_t[b][r0 : r0 + 64, mt, :]
            se = se_t[b]
            ph = pjt("ph")
            # se half 0 = block scores (pairs with vext[j] in the PV DR
            # matmul), half 1 = term scores (pairs with vext[5]).
            # zero-fill block region [0:640] (banks 0-1)
            nc.tensor.matmul(
                ph[:, 0:512], lhsT=zpair[:], rhs=xt_t[b][0:1, 0:2, 0:512],
                start=True, stop=False, perf_mode=DR,
            )
            nc.tensor.matmul(
                ph[:, 512:640], lhsT=zpair[:], rhs=xt_t[b][0:1, 0:2, 0:128],
                start=True, stop=False, perf_mode=DR,
            )
            # block-diagonal scores: per q-tile j a [128k x 128q] tile
            for j in range(NJ):
                for half in range(2):
                    c = 2 * j + half
                    nc.tensor.matmul(
                        ph[
                            half * 64 : half * 64 + 64,
                            j * P + half * 64 : j * P + half * 64 + 64,
                        ],
                        lhsT=KTh[:, c * L : (c + 1) * L],
                        rhs=QTh[:, c * L : (c + 1) * L],
                        start=False,
                        stop=(half == 1 and j in (3, 4)),
                    )
            # term scores^T [128 terms, 640 q] in cols [640:1280]
            nc.tensor.matmul(
                ph[:, 640:1024], lhsT=KTh[:, NQ:S], rhs=QTh[:, 0:384],
                start=True, stop=True,
            )
            nc.tensor.matmul(
                ph[:, 1024:1280], lhsT=KTh[:, NQ:S], rhs=QTh[:, 384:640],
                start=True, stop=True,
            )
            # exp (ACT): terms + blocks in ONE instruction (both APs uniform)
            nc.scalar.activation(
                se[:, hh, :, :, :].rearrange("p j two q -> p two j q"),
                ph[:, 0:1280].rearrange("p (two j q) -> p two j q", j=NJ, q=P),
                AF.Exp, scale=0.125,
            )

        def vsums(b):
            """corr'[c] = sum of v over candidate tokens NOT in pair(c)."""
            vef = ve_t[b].rearrange("p k h c -> p k (h c)")
            psc = smt("pscor")
            for c0, nlen in ((0, 512), (512, 256)):
                cs = slice(c0, c0 + nlen)
                for kp in range(2):
                    nc.tensor.matmul(
                        psc[0:64, cs],
                        lhsT=notG6[:, 2 * kp : 2 * kp + 2, :],
                        rhs=vef[:, 2 * kp : 2 * kp + 2, cs],
                        start=(kp == 0),
                        stop=False,
                        perf_mode=DR,
                    )
                nc.tensor.matmul(
                    psc[0:CDD, cs],
                    lhsT=notG6[:, 4, 0:CDD],
                    rhs=vef[:, 4, cs],
                    start=False,
                    stop=True,
                )
            corrE = mp.tile([CDD, ZC + NH], FP8, tag="corr", name=f"corrE_{b}")
            ce_t[b] = corrE
            nc.vector.tensor_copy(corrE[:, 0:ZC], psc[0:CDD, 0:ZC])
            # Z constant: 0.25 * 512 masked-out keys, via indall (0/1 rows)
            nc.gpsimd.memset(corrE[:, ZC : ZC + NH], 128.0)

        def pv_j(b, j, tag, last=False):
            se, vext, corrE, stg = se_t[b], ve_t[b], ce_t[b], sg_t[b]
            psc = pjt("pspv") if tag == "pj" else smt("pspv")
            # corr opens both banks' accumulation groups
            nc.tensor.matmul(
                psc[:, 0:512], lhsT=indall[:, j, :], rhs=corrE[:, 0:512],
                start=True, stop=False,
            )
            nc.tensor.matmul(
                psc[:, 512:768], lhsT=indall[:, j, :], rhs=corrE[:, 512:768],
                start=True, stop=False,
            )
            nc.tensor.matmul(
                psc[:, ZC : ZC + NH], lhsT=indall[:, j, :],
                rhs=corrE[:, ZC : ZC + NH],
                start=False, stop=False,
            )
            for hh in range(NH):
                nc.tensor.matmul(
                    psc[:, hh * DH : (hh + 1) * DH],
                    lhsT=se[:, hh, j, :, :],
                    rhs=vext[:, j : KT : 5 - j, hh, :],
                    start=False,
                    stop=(hh == 7),
                    perf_mode=DR,
                )
                nc.tensor.matmul(
                    psc[:, ZC + hh : ZC + hh + 1],
                    lhsT=se[:, hh, j, :, :],
                    rhs=onecol2[:],
                    start=False,
                    stop=(hh == NH - 1),
                    perf_mode=DR,
                )
            zr = mp.tile([P, NH], FP32, tag="zr", bufs=2, name="zr")
            nc.vector.reciprocal(zr[:], psc[:, ZC : ZC + NH])
            halves = ((0, 6), (6, 12)) if last else ((0, 12),)
            for lo, hi in halves:
                in0 = psc[:, lo * DH : hi * DH].rearrange(
                    "p (h c) -> p h c", c=DH
                )
                in1 = zr[:, lo:hi].rearrange("p (h o) -> p h o", o=1)
                bin0, bin1 = bass.broadcast_tensor_aps(in0, in1)
                nc.vector.scalar_tensor_tensor(
                    out=stg[:, j, lo * DH : hi * DH].rearrange(
                        "p (h c) -> p h c", c=DH
                    ),
                    in0=bin0,
                    scalar=ONECOL,
                    in1=bin1,
                    op0=ALU.mult,
                    op1=ALU.mult,
                )
                nc.sync.dma_start(
                    out=out[b].rearrange("(r p) h -> p r h", p=P)[
                        :, j, lo * DH : hi * DH
                    ],
                    in_=stg[:, j, lo * DH : hi * DH],
                )

        # ---------------- schedule ----------------
        # Scores lag projections by one m-tile so each group's score matmuls
        # overlap the NEXT group's projections instead of waiting on their
        # own Q/K copies. Batch 0 PV hides inside batch 1's loop.
        alloc_bufs(0)
        alloc_bufs(1)

        for mt in range(KT):
            qk_mt(0, mt)
            if mt == 0:
                # bvb[p, o] = bv[o] broadcast (rank-1); after first group so
                # the PE isn't blocked on the bvrow DMA at t=0
                psb = smt("psbv")
                nc.tensor.matmul(psb[:, 0:512], lhsT=onesrow[:], rhs=bvrow[0:1, 0:512], start=True, stop=True)
                nc.tensor.matmul(psb[:, 512:768], lhsT=onesrow[:], rhs=bvrow[0:1, 512:768], start=True, stop=True)
                nc.vector.tensor_copy(bvb[:, 0:512], psb[:, 0:512])
                nc.vector.tensor_copy(bvb[:, 512:768], psb[:, 512:768])
            else:
                v_tt(0, mt - 1)
                scores_h(0, 2 * (mt - 1))
                scores_h(0, 2 * (mt - 1) + 1)
        v_term(0)
        vsums(0)
        scores_h(0, 10)
        scores_h(0, 11)

        for mt in range(KT):
            qk_mt(1, mt)
            if mt < 5:
                v_tt(1, mt)
            if mt >= 1:
                scores_h(1, 2 * (mt - 1))
                scores_h(1, 2 * (mt - 1) + 1)
            if mt >= 2:
                pv_j(0, mt - 2, "pv")
        v_term(1)
        vsums(1)
        scores_h(1, 10)
        scores_h(1, 11)
        pv_j(0, 4, "pv")

        # batch 1 PV, double-buffered via the now-idle 3-bank pj tag
        for j in range(NJ):
            pv_j(1, j, "pj", last=(j == NJ - 1))


_CACHE = {}


def _get_program():
    if "nc" not in _CACHE:
        _CACHE["nc"] = _build_program()
    return _CACHE["nc"]


def _make_in_maps(inputs):
    f8 = ml_dtypes.float8_e4m3
    bf = ml_dtypes.bfloat16
    hs = np.asarray(inputs["hidden_states"], np.float32)
    hst = np.ascontiguousarray(hs.transpose(0, 2, 1))  # [B, H, S]
    wq = np.asarray(inputs["Wq"], np.float32).T * WS
    wk = np.asarray(inputs["Wk"], np.float32).T * WS
    wv = np.asarray(inputs["Wv"], np.float32).T
    in_common = {
        "wq8": np.ascontiguousarray(wq).astype(f8),
        "wk8": np.ascontiguousarray(wk).astype(f8),
        "wv8": np.ascontiguousarray(wv * WS).astype(f8),
        "wv16": np.ascontiguousarray(wv).astype(bf),
        "bq": np.asarray(inputs["bq"], np.float32),
        "bk": np.asarray(inputs["bk"], np.float32),
        "bv16": np.asarray(inputs["bv"], np.float32).astype(bf),
    }
    x8 = hst.astype(f8)
    # xt16 packed [BL, P, KT, T] so the DMA's contiguous runs are 1536B
    xm = (
        hst[:, :, NQ:]
        .reshape(B, KT, P, T)
        .transpose(0, 2, 1, 3)
        .astype(bf)
    )
    return [
        {
            "x8": x8[i * BL : (i + 1) * BL],
            "xt16": np.ascontiguousarray(xm[i * BL : (i + 1) * BL]),
            **in_common,
        }
        for i in range(NCORES)
    ]


def kernel(**inputs) -> np.ndarray:
    in_maps = _make_in_maps(inputs)
    nc = _get_program()
    res = run_bass_kernel_spmd(nc, in_maps, list(range(NCORES)))
    return np.concatenate([res.results[i]["out"] for i in range(NCORES)], axis=0)


# revision 6
# speedup vs baseline: 1.7334x; 1.0191x over previous
"""Trainium2 Bass kernel for nn_BertSelfAttention_79577154060613 (fp8).

Block-sparse BERT self-attention, data-parallel over batch across 8 cores
(2 batches/core). Cost-model-guided redesign of the bf16 baseline:

- All projections run as fp8e4m3 DoubleRow matmuls: 2 contraction k-tiles
  packed per instruction at 0.5 cycles/row -> 4x cheaper than bf16. Weights
  are pre-scaled by 16 on the host so w*16 lands in fp8's normal range; the
  PSUM->SBUF copies divide by 16 (free via the copy's scale port).
- The V projection of the 128 term tokens runs in bf16 (the reference output
  passes those rows through untouched, so they set the error floor). All
  other fp8 error sources only perturb softmax-averaged context and stay
  ~1e-3 absolute.
- Per head, term scores ([128 terms x 640 q]) and block scores land in one
  3-bank PSUM tile; block scores are computed as 5 block-diagonal [128x128]
  key-x-query tiles whose off-diagonal quadrants are zero-filled by two
  rank-1 DoubleRow matmuls. exp(0)=1 garbage in those quadrants contributes
  exactly sum(v over the sibling block), which the correction term absorbs
  by excluding the whole block PAIR instead of just the own block:
    ctx*Z = sum_{k in block|terms} e^s v_k + 1*Vsum_sibling + corr'(c)
    corr'(c) = sum_{c' not in pair(c)} Vsum_c'   (rank-10 matmul vs vext)
  This allows ONE exp instruction per head for all block scores.
- PV is a single DoubleRow matmul per (q-tile, head): contraction half A =
  128 exp'ed term scores, half B = the 128-row block-diagonal exp'ed block
  scores; rhs halves are vext[term-tile] and vext[j-tile]. The softmax
  denominator is accumulated by rank-1 DoubleRow matmuls (se @ 0.25-column)
  into columns [768:780] of the same PSUM pair, with the corr matmul
  contributing the 0.25*512 constant from masked-out keys; the heads' 64-col
  context slices are gapless so the divide is ONE DVE op per q-tile.
- Output is staged in SBUF [128, 6, 768] fp32 and DMA'd per 128-row slice
  (6 DMAs/batch). Phases are software-pipelined: batch0 PV hides inside
  batch1's projection loop; batch1 PV double-buffers via the then-idle
  3-bank PSUM tag.
"""

import numpy as np
import ml_dtypes

import concourse.bass as bass
import concourse.mybir as mybir
import concourse.tile as tile
from concourse import bacc
from concourse.bass_utils import run_bass_kernel_spmd

B, CDD, L, T, H, NH = 16, 10, 64, 128, 768, 12
DH = H // NH  # 64
S = CDD * L + T  # 768
NQ = CDD * L  # 640
P = 128
NCORES = 8
BL = B // NCORES  # 2
KT = H // P  # 6
FP32 = mybir.dt.float32
BF16 = mybir.dt.bfloat16
FP8 = mybir.dt.float8e4
AF = mybir.ActivationFunctionType
ALU = mybir.AluOpType
DR = mybir.MatmulPerfMode.DoubleRow
ONECOL = 0.25  # Z-column scale (keeps the corr Z constant fp8-exact: 128)
WS = 16.0  # host-side weight scale
IWS = 1.0 / WS
NJ = NQ // P  # 5 q-tiles
ZC = NH * DH  # 768: column where the Z region starts in psc


def _build_program():
    nc = bacc.Bacc(
        "TRN2", target_bir_lowering=False, debug=False, num_devices=NCORES
    )
    x8 = nc.dram_tensor("x8", [BL, H, S], FP8, kind="ExternalInput").ap()
    xt16 = nc.dram_tensor("xt16", [BL, P, KT, T], BF16, kind="ExternalInput").ap()
    wq8 = nc.dram_tensor("wq8", [H, H], FP8, kind="ExternalInput").ap()
    wk8 = nc.dram_tensor("wk8", [H, H], FP8, kind="ExternalInput").ap()
    wv8 = nc.dram_tensor("wv8", [H, H], FP8, kind="ExternalInput").ap()
    wv16 = nc.dram_tensor("wv16", [H, H], BF16, kind="ExternalInput").ap()
    bq = nc.dram_tensor("bq", [H], FP32, kind="ExternalInput").ap()
    bk = nc.dram_tensor("bk", [H], FP32, kind="ExternalInput").ap()
    bv16 = nc.dram_tensor("bv16", [H], BF16, kind="ExternalInput").ap()
    out = nc.dram_tensor("out", [BL, S, H], FP32, kind="ExternalOutput").ap()

    with tile.TileContext(nc) as tc:
        _emit(tc, nc, x8, xt16, wq8, wk8, wv8, wv16, bq, bk, bv16, out)
    nc.compile()
    return nc


def _emit(tc, nc, x8, xt16, wq8, wk8, wv8, wv16, bq, bk, bv16, out):
    from contextlib import ExitStack

    ctx = ExitStack()
    with ctx:
        cp = ctx.enter_context(tc.tile_pool(name="consts", bufs=1))
        wp = ctx.enter_context(tc.tile_pool(name="weights", bufs=1))
        xp = ctx.enter_context(tc.tile_pool(name="xin", bufs=2))
        qp = ctx.enter_context(tc.tile_pool(name="qkv", bufs=2))
        sp = ctx.enter_context(tc.tile_pool(name="sexp", bufs=2))
        op = ctx.enter_context(tc.tile_pool(name="ostg", bufs=2))
        mp = ctx.enter_context(tc.tile_pool(name="small", bufs=2))
        pa = ctx.enter_context(tc.tile_pool(name="ps", bufs=1, space="PSUM"))

        def pjt(name):
            return pa.tile(
                [P, 1536], FP32, tag="pj", bufs=2, name=name,
                padded_shape=[P, 1536],
            )

        def smt(name):
            return pa.tile(
                [P, 1024], FP32, tag="sm", bufs=1, name=name,
                padded_shape=[P, 1024],
            )

        # ---------------- input DMAs (critical-path order) ----------------
        # first Q m-tile needs wq cols [0:128] and xt kt-pair 0 only: split
        # those DMAs so the PE can start ~1.5us in
        wq_sb = wp.tile([P, KT, H], FP8, name="wq8sb")
        wq_r = wq8.rearrange("(k p) o -> p k o", p=P)
        nc.sync.dma_start(out=wq_sb[:, :, 0:P], in_=wq_r[:, :, 0:P])
        xt_t, xm_t = [], []
        for b in range(BL):
            xt_t.append(xp.tile([P, KT, S], FP8, tag="xt", name=f"xt{b}"))
            xm_t.append(xp.tile([P, KT, T], BF16, tag="xm", name=f"xm{b}"))
        x0_r = x8[0].rearrange("(k p) s -> p k s", p=P)
        nc.sync.dma_start(out=xt_t[0][:, 0:2, :], in_=x0_r[:, 0:2, :])
        nc.sync.dma_start(out=xt_t[0][:, 2:6, :], in_=x0_r[:, 2:6, :])
        wk_sb = wp.tile([P, KT, H], FP8, name="wk8sb")
        wk_r = wk8.rearrange("(k p) o -> p k o", p=P)
        nc.sync.dma_start(out=wk_sb[:, :, 0:P], in_=wk_r[:, :, 0:P])
        bcq = cp.tile([P, KT], FP32, name="bcq")
        nc.sync.dma_start(out=bcq[:], in_=bq.rearrange("(t p) -> p t", p=P))
        bck = cp.tile([P, KT], FP32, name="bck")
        nc.sync.dma_start(out=bck[:], in_=bk.rearrange("(t p) -> p t", p=P))
        nc.sync.dma_start(out=wq_sb[:, :, P:H], in_=wq_r[:, :, P:H])
        nc.sync.dma_start(out=wk_sb[:, :, P:H], in_=wk_r[:, :, P:H])
        wv_sb = wp.tile([P, KT, H], FP8, name="wv8sb")
        nc.sync.dma_start(out=wv_sb[:], in_=wv8.rearrange("(k p) o -> p k o", p=P))
        bvrow = cp.tile([1, H], BF16, name="bvrow")
        nc.sync.dma_start(out=bvrow[:], in_=bv16[None, :])
        nc.sync.dma_start(out=xt_t[1][:], in_=x8[1].rearrange("(k p) s -> p k s", p=P))
        wv16_sb = wp.tile([P, KT, H], BF16, name="wv16sb")
        nc.sync.dma_start(out=wv16_sb[:], in_=wv16.rearrange("(k p) o -> p k o", p=P))
        nc.sync.dma_start(out=xm_t[0][:], in_=xt16[0])
        nc.sync.dma_start(out=xm_t[1][:], in_=xt16[1])

        # ---------------- constants (Pool) ----------------
        onesrow = cp.tile([1, P], FP8, name="onesrow")
        nc.gpsimd.memset(onesrow[:], 1.0)
        zpair = cp.tile([1, 2, P], FP8, name="zpair")
        nc.gpsimd.memset(zpair[:], 0.0)
        onecol2 = cp.tile([P, 2, 1], FP8, name="onecol2")
        nc.gpsimd.memset(onecol2[:], ONECOL)
        # notG6[p, kt, c] = 0 if block c is in tile kt's pair else 1.
        # Inner dim padded to 64 so dual-fp8 LdWeights half-stride is aligned
        # (cols 10:64 are zero -> psum rows 10:64 unused).
        notG6 = cp.tile([P, KT, 64], FP8, name="notG6")
        nc.gpsimd.memset(notG6[:], 0.0)
        nc.gpsimd.memset(notG6[:, :, 0:CDD], 1.0)
        for kt in range(5):
            nc.gpsimd.memset(notG6[:, kt, 2 * kt : 2 * kt + 2], 0.0)
        # indall[c, j, q] = 1 iff query q of tile j belongs to block c,
        # i.e. c - 2j - (q // 64) == 0
        indall = cp.tile([CDD, NJ, P], FP8, name="indall")
        nc.gpsimd.memset(indall[:], 1.0)
        nc.gpsimd.affine_select(
            out=indall.rearrange("c j (h q) -> c j h q", q=64),
            in_=indall.rearrange("c j (h q) -> c j h q", q=64),
            compare_op=ALU.is_equal,
            fill=0.0,
            base=0,
            pattern=[[-2, NJ], [-1, 2], [0, 64]],
            channel_multiplier=1,
        )
        bvb = cp.tile([P, H], FP32, name="bvb")

        qt_t = [None] * BL
        kt_t = [None] * BL
        ve_t = [None] * BL
        se_t = [None] * BL
        ce_t = [None] * BL
        sg_t = [None] * BL

        wq8_z = None  # zero-fill rhs uses xt slices (DMA'd first)

        def alloc_bufs(b):
            qt_t[b] = qp.tile([P, KT, NQ], FP8, tag="qt", name=f"qt8_{b}")
            kt_t[b] = qp.tile([P, KT, S], FP8, tag="kt", name=f"kt8_{b}")
            ve_t[b] = qp.tile([P, KT, NH, DH], FP8, tag="ve", name=f"vext_{b}")
            se_t[b] = sp.tile([P, NH, NJ, 2, P], FP8, tag="se", name=f"se_{b}")
            sg_t[b] = op.tile([P, KT, H], FP32, tag="stg", name=f"stg_{b}")

        def qk_mt(b, mt):
            """Q and K projections for one m-tile, sharing a 3-bank psum:
            Q at [0:512]+[512:640], K at [640:1024]+[1024:1408]."""
            ms = slice(mt * P, (mt + 1) * P)
            ps = pjt("psqk")
            for c0, nlen in ((0, 512), (512, 128)):
                for kp in range(3):
                    nc.tensor.matmul(
                        ps[:, c0 : c0 + nlen],
                        lhsT=wq_sb[:, 2 * kp : 2 * kp + 2, ms],
                        rhs=xt_t[b][:, 2 * kp : 2 * kp + 2, c0 : c0 + nlen],
                        start=(kp == 0),
                        stop=(kp == 2),
                        perf_mode=DR,
                    )
            for c0, nlen in ((640, 384), (1024, 384)):
                for kp in range(3):
                    nc.tensor.matmul(
                        ps[:, c0 : c0 + nlen],
                        lhsT=wk_sb[:, 2 * kp : 2 * kp + 2, ms],
                        rhs=xt_t[b][:, 2 * kp : 2 * kp + 2, c0 - 640 : c0 - 640 + nlen],
                        start=(kp == 0),
                        stop=(kp == 2),
                        perf_mode=DR,
                    )
            if mt < 5 or b == 1:  # balance: most Q copies on ACT
                nc.scalar.activation(
                    qt_t[b][:, mt, :], ps[:, 0:NQ], AF.Identity,
                    bias=bcq[:, mt : mt + 1], scale=IWS,
                )
            else:
                nc.vector.tensor_scalar(
                    qt_t[b][:, mt, :], ps[:, 0:NQ], IWS, bcq[:, mt : mt + 1],
                    op0=ALU.mult, op1=ALU.add,
                )
            nc.vector.tensor_scalar(
                kt_t[b][:, mt, :], ps[:, 640 : 640 + S], IWS, bck[:, mt : mt + 1],
                op0=ALU.mult, op1=ALU.add,
            )

        def v_tt(b, tt):
            """Candidate-token V tile (fp8)."""
            ts = slice(tt * P, (tt + 1) * P)
            psv = smt("psv")
            for c0, nlen in ((0, 512), (512, 256)):
                for kp in range(3):
                    nc.tensor.matmul(
                        psv[:, c0 : c0 + nlen],
                        lhsT=xt_t[b][:, 2 * kp : 2 * kp + 2, ts],
                        rhs=wv_sb[:, 2 * kp : 2 * kp + 2, c0 : c0 + nlen],
                        start=(kp == 0),
                        stop=(kp == 2),
                        perf_mode=DR,
                    )
            nc.vector.scalar_tensor_tensor(
                out=ve_t[b][:, tt, :, :],
                in0=psv[:, 0:H].rearrange("p (h c) -> p h c", c=DH),
                scalar=IWS,
                in1=bvb.rearrange("p (h c) -> p h c", c=DH),
                op0=ALU.mult,
                op1=ALU.add,
            )

        def v_term(b):
            """Term-token V in bf16 (output passthrough accuracy)."""
            psv = smt("psvt")
            for c0, nlen in ((0, 512), (512, 256)):
                for kt in range(KT):
                    nc.tensor.matmul(
                        psv[:, c0 : c0 + nlen],
                        lhsT=xm_t[b][:, kt, :],
                        rhs=wv16_sb[:, kt, c0 : c0 + nlen],
                        start=(kt == 0),
                        stop=(kt == KT - 1),
                    )
            stg = sg_t[b]
            nc.vector.tensor_tensor(
                out=stg[:, 5, :], in0=psv[:, 0:H], in1=bvb[:], op=ALU.add
            )
            nc.gpsimd.tensor_copy(
                ve_t[b][:, 5, :, :],
                stg[:, 5, :].rearrange("p (h c) -> p h c", c=DH),
            )
            nc.sync.dma_start(
                out=out[b].rearrange("(r p) h -> p r h", p=P)[:, 5, :],
                in_=stg[:, 5, :],
            )

        def scores_h(b, hh):
            mt, hl = hh // 2, hh % 2
            r0 = hl * 64
            KTh = kt_t[b][r0 : r0 + 64, mt, :]
            QTh = qt_t[b][r0 : r0 + 64, mt, :]
            se = se_t[b]
            ph = pjt("ph")
            # se half 0 = block scores (pairs with vext[j] in the PV DR
            # matmul), half 1 = term scores (pairs with vext[5]).
            # zero-fill block region [0:640] (banks 0-1)
            nc.tensor.matmul(
                ph[:, 0:512], lhsT=zpair[:], rhs=xt_t[b][0:1, 0:2, 0:512],
                start=True, stop=False, perf_mode=DR,
            )
            nc.tensor.matmul(
                ph[:, 512:640], lhsT=zpair[:], rhs=xt_t[b][0:1, 0:2, 0:128],
                start=True, stop=False, perf_mode=DR,
            )
            # block-diagonal scores: per q-tile j a [128k x 128q] tile
            for j in range(NJ):
                for half in range(2):
                    c = 2 * j + half
                    nc.tensor.matmul(
                        ph[
                            half * 64 : half * 64 + 64,
                            j * P + half * 64 : j * P + half * 64 + 64,
                        ],
                        lhsT=KTh[:, c * L : (c + 1) * L],
                        rhs=QTh[:, c * L : (c + 1) * L],
                        start=False,
                        stop=(half == 1 and j in (3, 4)),
                    )
            # term scores^T [128 terms, 640 q] in cols [640:1280]
            nc.tensor.matmul(
                ph[:, 640:1024], lhsT=KTh[:, NQ:S], rhs=QTh[:, 0:384],
                start=True, stop=True,
            )
            nc.tensor.matmul(
                ph[:, 1024:1280], lhsT=KTh[:, NQ:S], rhs=QTh[:, 384:640],
                start=True, stop=True,
            )
            # exp (ACT): terms + blocks in ONE instruction (both APs uniform)
            nc.scalar.activation(
                se[:, hh, :, :, :].rearrange("p j two q -> p two j q"),
                ph[:, 0:1280].rearrange("p (two j q) -> p two j q", j=NJ, q=P),
                AF.Exp, scale=0.125,
            )

        def vsums(b):
            """corr'[c] = sum of v over candidate tokens NOT in pair(c)."""
            vef = ve_t[b].rearrange("p k h c -> p k (h c)")
            psc = smt("pscor")
            for c0, nlen in ((0, 512), (512, 256)):
                cs = slice(c0, c0 + nlen)
                for kp in range(2):
                    nc.tensor.matmul(
                        psc[0:64, cs],
                        lhsT=notG6[:, 2 * kp : 2 * kp + 2, :],
                        rhs=vef[:, 2 * kp : 2 * kp + 2, cs],
                        start=(kp == 0),
                        stop=False,
                        perf_mode=DR,
                    )
                nc.tensor.matmul(
                    psc[0:CDD, cs],
                    lhsT=notG6[:, 4, 0:CDD],
                    rhs=vef[:, 4, cs],
                    start=False,
                    stop=True,
                )
            corrE = mp.tile([CDD, ZC + NH], FP8, tag="corr", name=f"corrE_{b}")
            ce_t[b] = corrE
            nc.vector.tensor_copy(corrE[:, 0:ZC], psc[0:CDD, 0:ZC])
            # Z constant: 0.25 * 512 masked-out keys, via indall (0/1 rows)
            nc.gpsimd.memset(corrE[:, ZC : ZC + NH], 128.0)

        def pv_j(b, j, tag, last=False):
            se, vext, corrE, stg = se_t[b], ve_t[b], ce_t[b], sg_t[b]
            psc = pjt("pspv") if tag == "pj" else smt("pspv")
            # corr opens both banks' accumulation groups
            nc.tensor.matmul(
                psc[:, 0:512], lhsT=indall[:, j, :], rhs=corrE[:, 0:512],
                start=True, stop=False,
            )
            nc.tensor.matmul(
                psc[:, 512:768], lhsT=indall[:, j, :], rhs=corrE[:, 512:768],
                start=True, stop=False,
            )
            nc.tensor.matmul(
                psc[:, ZC : ZC + NH], lhsT=indall[:, j, :],
                rhs=corrE[:, ZC : ZC + NH],
                start=False, stop=False,
            )
            for hh in range(NH):
                nc.tensor.matmul(
                    psc[:, hh * DH : (hh + 1) * DH],
                    lhsT=se[:, hh, j, :, :],
                    rhs=vext[:, j : KT : 5 - j, hh, :],
                    start=False,
                    stop=(hh == 7),
                    perf_mode=DR,
                )
                nc.tensor.matmul(
                    psc[:, ZC + hh : ZC + hh + 1],
                    lhsT=se[:, hh, j, :, :],
                    rhs=onecol2[:],
                    start=False,
                    stop=(hh == NH - 1),
                    perf_mode=DR,
                )
            zr = mp.tile([P, NH], FP32, tag="zr", bufs=2, name="zr")
            nc.vector.reciprocal(zr[:], psc[:, ZC : ZC + NH])
            halves = ((0, 6), (6, 12)) if last else ((0, 12),)
            for lo, hi in halves:
                in0 = psc[:, lo * DH : hi * DH].rearrange(
                    "p (h c) -> p h c", c=DH
                )
                in1 = zr[:, lo:hi].rearrange("p (h o) -> p h o", o=1)
                bin0, bin1 = bass.broadcast_tensor_aps(in0, in1)
                nc.vector.scalar_tensor_tensor(
                    out=stg[:, j, lo * DH : hi * DH].rearrange(
                        "p (h c) -> p h c", c=DH
                    ),
                    in0=bin0,
                    scalar=ONECOL,
                    in1=bin1,
                    op0=ALU.mult,
                    op1=ALU.mult,
                )
                nc.sync.dma_start(
                    out=out[b].rearrange("(r p) h -> p r h", p=P)[
                        :, j, lo * DH : hi * DH
                    ],
                    in_=stg[:, j, lo * DH : hi * DH],
                )

        # ---------------- schedule ----------------
        # Scores lag projections by one m-tile so each group's score matmuls
        # overlap the NEXT group's projections instead of waiting on their
        # own Q/K copies. Batch 0 PV hides inside batch 1's loop.
        alloc_bufs(0)
        alloc_bufs(1)

        for mt in range(KT):
            qk_mt(0, mt)
            if mt == 0:
                # bvb[p, o] = bv[o] broadcast (rank-1); after first group so
                # the PE isn't blocked on the bvrow DMA at t=0
                psb = smt("psbv")
                nc.tensor.matmul(psb[:, 0:512], lhsT=onesrow[:], rhs=bvrow[0:1, 0:512], start=True, stop=True)
                nc.tensor.matmul(psb[:, 512:768], lhsT=onesrow[:], rhs=bvrow[0:1, 512:768], start=True, stop=True)
                nc.vector.tensor_copy(bvb[:, 0:512], psb[:, 0:512])
                nc.vector.tensor_copy(bvb[:, 512:768], psb[:, 512:768])
            else:
                v_tt(0, mt - 1)
                scores_h(0, 2 * (mt - 1))
                scores_h(0, 2 * (mt - 1) + 1)
        v_term(0)
        vsums(0)
        scores_h(0, 10)
        scores_h(0, 11)

        for mt in range(KT):
            qk_mt(1, mt)
            if mt >= 2:
                pv_j(0, mt - 2, "pv")
            if mt < 5:
                v_tt(1, mt)
            if mt >= 1:
                scores_h(1, 2 * (mt - 1))
                scores_h(1, 2 * (mt - 1) + 1)
        v_term(1)
        vsums(1)
        scores_h(1, 10)
        scores_h(1, 11)
        pv_j(0, 4, "pv")

        # batch 1 PV, double-buffered via the now-idle 3-bank pj tag
        for j in range(NJ):
            pv_j(1, j, "pj", last=(j == NJ - 1))


_CACHE = {}


def _get_program():
    if "nc" not in _CACHE:
        _CACHE["nc"] = _build_program()
    return _CACHE["nc"]


def _make_in_maps(inputs):
    f8 = ml_dtypes.float8_e4m3
    bf = ml_dtypes.bfloat16
    hs = np.asarray(inputs["hidden_states"], np.float32)
    hst = np.ascontiguousarray(hs.transpose(0, 2, 1))  # [B, H, S]
    wq = np.asarray(inputs["Wq"], np.float32).T * WS
    wk = np.asarray(inputs["Wk"], np.float32).T * WS
    wv = np.asarray(inputs["Wv"], np.float32).T
    in_common = {
        "wq8": np.ascontiguousarray(wq).astype(f8),
        "wk8": np.ascontiguousarray(wk).astype(f8),
        "wv8": np.ascontiguousarray(wv * WS).astype(f8),
        "wv16": np.ascontiguousarray(wv).astype(bf),
        "bq": np.asarray(inputs["bq"], np.float32),
        "bk": np.asarray(inputs["bk"], np.float32),
        "bv16": np.asarray(inputs["bv"], np.float32).astype(bf),
    }
    x8 = hst.astype(f8)
    # xt16 packed [BL, P, KT, T] so the DMA's contiguous runs are 1536B
    xm = (
        hst[:, :, NQ:]
        .reshape(B, KT, P, T)
        .transpose(0, 2, 1, 3)
        .astype(bf)
    )
    return [
        {
            "x8": x8[i * BL : (i + 1) * BL],
            "xt16": np.ascontiguousarray(xm[i * BL : (i + 1) * BL]),
            **in_common,
        }
        for i in range(NCORES)
    ]


def kernel(**inputs) -> np.ndarray:
    in_maps = _make_in_maps(inputs)
    nc = _get_program()
    res = run_bass_kernel_spmd(nc, in_maps, list(range(NCORES)))
    return np.concatenate([res.results[i]["out"] for i in range(NCORES)], axis=0)


# revision 7
# speedup vs baseline: 1.7384x; 1.0029x over previous
"""Trainium2 Bass kernel for nn_BertSelfAttention_79577154060613 (fp8).

Block-sparse BERT self-attention, data-parallel over batch across 8 cores
(2 batches/core). Cost-model-guided redesign of the bf16 baseline:

- All projections run as fp8e4m3 DoubleRow matmuls: 2 contraction k-tiles
  packed per instruction at 0.5 cycles/row -> 4x cheaper than bf16. Weights
  are pre-scaled by 16 on the host so w*16 lands in fp8's normal range; the
  PSUM->SBUF copies divide by 16 (free via the copy's scale port).
- The V projection of the 128 term tokens runs in bf16 (the reference output
  passes those rows through untouched, so they set the error floor). All
  other fp8 error sources only perturb softmax-averaged context and stay
  ~1e-3 absolute.
- Per head, term scores ([128 terms x 640 q]) and block scores land in one
  3-bank PSUM tile; block scores are computed as 5 block-diagonal [128x128]
  key-x-query tiles whose off-diagonal quadrants are zero-filled by two
  rank-1 DoubleRow matmuls. exp(0)=1 garbage in those quadrants contributes
  exactly sum(v over the sibling block), which the correction term absorbs
  by excluding the whole block PAIR instead of just the own block:
    ctx*Z = sum_{k in block|terms} e^s v_k + 1*Vsum_sibling + corr'(c)
    corr'(c) = sum_{c' not in pair(c)} Vsum_c'   (rank-10 matmul vs vext)
  This allows ONE exp instruction per head for all block scores.
- PV is a single DoubleRow matmul per (q-tile, head): contraction half A =
  128 exp'ed term scores, half B = the 128-row block-diagonal exp'ed block
  scores; rhs halves are vext[term-tile] and vext[j-tile]. The softmax
  denominator is accumulated by rank-1 DoubleRow matmuls (se @ 0.25-column)
  into columns [768:780] of the same PSUM pair, with the corr matmul
  contributing the 0.25*512 constant from masked-out keys; the heads' 64-col
  context slices are gapless so the divide is ONE DVE op per q-tile.
- Output is staged in SBUF [128, 6, 768] fp32 and DMA'd per 128-row slice
  (6 DMAs/batch). Phases are software-pipelined: batch0 PV hides inside
  batch1's projection loop; batch1 PV double-buffers via the then-idle
  3-bank PSUM tag.
"""

import numpy as np
import ml_dtypes

import concourse.bass as bass
import concourse.mybir as mybir
import concourse.tile as tile
from concourse import bacc
from concourse.bass_utils import run_bass_kernel_spmd

B, CDD, L, T, H, NH = 16, 10, 64, 128, 768, 12
DH = H // NH  # 64
S = CDD * L + T  # 768
NQ = CDD * L  # 640
P = 128
NCORES = 8
BL = B // NCORES  # 2
KT = H // P  # 6
FP32 = mybir.dt.float32
BF16 = mybir.dt.bfloat16
FP8 = mybir.dt.float8e4
AF = mybir.ActivationFunctionType
ALU = mybir.AluOpType
DR = mybir.MatmulPerfMode.DoubleRow
ONECOL = 0.25  # Z-column scale (keeps the corr Z constant fp8-exact: 128)
WS = 16.0  # host-side weight scale
IWS = 1.0 / WS
NJ = NQ // P  # 5 q-tiles
ZC = NH * DH  # 768: column where the Z region starts in psc


def _build_program():
    nc = bacc.Bacc(
        "TRN2", target_bir_lowering=False, debug=False, num_devices=NCORES
    )
    x8 = nc.dram_tensor("x8", [BL, H, S], FP8, kind="ExternalInput").ap()
    xt16 = nc.dram_tensor("xt16", [BL, P, KT, T], BF16, kind="ExternalInput").ap()
    wq8 = nc.dram_tensor("wq8", [H, H], FP8, kind="ExternalInput").ap()
    wk8 = nc.dram_tensor("wk8", [H, H], FP8, kind="ExternalInput").ap()
    wv8 = nc.dram_tensor("wv8", [H, H], FP8, kind="ExternalInput").ap()
    wv16 = nc.dram_tensor("wv16", [H, H], BF16, kind="ExternalInput").ap()
    bq = nc.dram_tensor("bq", [H], FP32, kind="ExternalInput").ap()
    bk = nc.dram_tensor("bk", [H], FP32, kind="ExternalInput").ap()
    bv16 = nc.dram_tensor("bv16", [H], BF16, kind="ExternalInput").ap()
    out = nc.dram_tensor("out", [BL, S, H], FP32, kind="ExternalOutput").ap()

    with tile.TileContext(nc) as tc:
        _emit(tc, nc, x8, xt16, wq8, wk8, wv8, wv16, bq, bk, bv16, out)
    nc.compile()
    return nc


def _emit(tc, nc, x8, xt16, wq8, wk8, wv8, wv16, bq, bk, bv16, out):
    from contextlib import ExitStack

    ctx = ExitStack()
    with ctx:
        cp = ctx.enter_context(tc.tile_pool(name="consts", bufs=1))
        wp = ctx.enter_context(tc.tile_pool(name="weights", bufs=1))
        xp = ctx.enter_context(tc.tile_pool(name="xin", bufs=2))
        qp = ctx.enter_context(tc.tile_pool(name="qkv", bufs=2))
        sp = ctx.enter_context(tc.tile_pool(name="sexp", bufs=2))
        op = ctx.enter_context(tc.tile_pool(name="ostg", bufs=2))
        mp = ctx.enter_context(tc.tile_pool(name="small", bufs=2))
        pa = ctx.enter_context(tc.tile_pool(name="ps", bufs=1, space="PSUM"))

        def pjt(name):
            return pa.tile(
                [P, 1536], FP32, tag="pj", bufs=2, name=name,
                padded_shape=[P, 1536],
            )

        def smt(name):
            return pa.tile(
                [P, 1024], FP32, tag="sm", bufs=1, name=name,
                padded_shape=[P, 1024],
            )

        # ---------------- input DMAs (critical-path order) ----------------
        # first Q m-tile needs wq cols [0:128] and xt kt-pair 0 only: split
        # those DMAs so the PE can start ~1.5us in
        wq_sb = wp.tile([P, KT, H], FP8, name="wq8sb")
        wq_r = wq8.rearrange("(k p) o -> p k o", p=P)
        nc.sync.dma_start(out=wq_sb[:, :, 0:P], in_=wq_r[:, :, 0:P])
        xt_t, xm_t = [], []
        for b in range(BL):
            xt_t.append(xp.tile([P, KT, S], FP8, tag="xt", name=f"xt{b}"))
            xm_t.append(xp.tile([P, KT, T], BF16, tag="xm", name=f"xm{b}"))
        x0_r = x8[0].rearrange("(k p) s -> p k s", p=P)
        nc.sync.dma_start(out=xt_t[0][:, 0:2, :], in_=x0_r[:, 0:2, :])
        nc.sync.dma_start(out=xt_t[0][:, 2:6, :], in_=x0_r[:, 2:6, :])
        wk_sb = wp.tile([P, KT, H], FP8, name="wk8sb")
        wk_r = wk8.rearrange("(k p) o -> p k o", p=P)
        nc.sync.dma_start(out=wk_sb[:, :, 0:P], in_=wk_r[:, :, 0:P])
        bcq = cp.tile([P, KT], FP32, name="bcq")
        nc.sync.dma_start(out=bcq[:], in_=bq.rearrange("(t p) -> p t", p=P))
        bck = cp.tile([P, KT], FP32, name="bck")
        nc.sync.dma_start(out=bck[:], in_=bk.rearrange("(t p) -> p t", p=P))
        nc.sync.dma_start(out=wq_sb[:, :, P:H], in_=wq_r[:, :, P:H])
        nc.sync.dma_start(out=wk_sb[:, :, P:H], in_=wk_r[:, :, P:H])
        wv_sb = wp.tile([P, KT, H], FP8, name="wv8sb")
        nc.sync.dma_start(out=wv_sb[:], in_=wv8.rearrange("(k p) o -> p k o", p=P))
        bvrow = cp.tile([1, H], BF16, name="bvrow")
        nc.sync.dma_start(out=bvrow[:], in_=bv16[None, :])
        nc.sync.dma_start(out=xt_t[1][:], in_=x8[1].rearrange("(k p) s -> p k s", p=P))
        wv16_sb = wp.tile([P, KT, H], BF16, name="wv16sb")
        nc.sync.dma_start(out=wv16_sb[:], in_=wv16.rearrange("(k p) o -> p k o", p=P))
        nc.sync.dma_start(out=xm_t[0][:], in_=xt16[0])
        nc.sync.dma_start(out=xm_t[1][:], in_=xt16[1])

        # ---------------- constants (Pool) ----------------
        onesrow = cp.tile([1, P], FP8, name="onesrow")
        nc.gpsimd.memset(onesrow[:], 1.0)
        zpair = cp.tile([1, 2, P], FP8, name="zpair")
        nc.gpsimd.memset(zpair[:], 0.0)
        onecol2 = cp.tile([P, 2, 1], FP8, name="onecol2")
        nc.gpsimd.memset(onecol2[:], ONECOL)
        # notG6[p, kt, c] = 0 if block c is in tile kt's pair else 1.
        # Inner dim padded to 64 so dual-fp8 LdWeights half-stride is aligned
        # (cols 10:64 are zero -> psum rows 10:64 unused).
        notG6 = cp.tile([P, KT, 64], FP8, name="notG6")
        nc.gpsimd.memset(notG6[:], 0.0)
        nc.gpsimd.memset(notG6[:, :, 0:CDD], 1.0)
        for kt in range(5):
            nc.gpsimd.memset(notG6[:, kt, 2 * kt : 2 * kt + 2], 0.0)
        # indall[c, j, q] = 1 iff query q of tile j belongs to block c,
        # i.e. c - 2j - (q // 64) == 0
        indall = cp.tile([CDD, NJ, P], FP8, name="indall")
        nc.gpsimd.memset(indall[:], 1.0)
        nc.gpsimd.affine_select(
            out=indall.rearrange("c j (h q) -> c j h q", q=64),
            in_=indall.rearrange("c j (h q) -> c j h q", q=64),
            compare_op=ALU.is_equal,
            fill=0.0,
            base=0,
            pattern=[[-2, NJ], [-1, 2], [0, 64]],
            channel_multiplier=1,
        )
        bvb = cp.tile([P, H], FP32, name="bvb")

        qt_t = [None] * BL
        kt_t = [None] * BL
        ve_t = [None] * BL
        se_t = [None] * BL
        ce_t = [None] * BL
        sg_t = [None] * BL

        wq8_z = None  # zero-fill rhs uses xt slices (DMA'd first)

        def alloc_bufs(b):
            qt_t[b] = qp.tile([P, KT, NQ], FP8, tag="qt", name=f"qt8_{b}")
            kt_t[b] = qp.tile([P, KT, S], FP8, tag="kt", name=f"kt8_{b}")
            ve_t[b] = qp.tile([P, KT, NH, DH], FP8, tag="ve", name=f"vext_{b}")
            se_t[b] = sp.tile([P, NH, NJ, 2, P], FP8, tag="se", name=f"se_{b}")
            sg_t[b] = op.tile([P, KT, H], FP32, tag="stg", name=f"stg_{b}")

        def qk_mt(b, mt):
            """Q and K projections for one m-tile, sharing a 3-bank psum:
            Q at [0:512]+[512:640], K at [640:1024]+[1024:1408]."""
            ms = slice(mt * P, (mt + 1) * P)
            ps = pjt("psqk")

            def qmm(c0, nlen, kp):
                nc.tensor.matmul(
                    ps[:, c0 : c0 + nlen],
                    lhsT=wq_sb[:, 2 * kp : 2 * kp + 2, ms],
                    rhs=xt_t[b][:, 2 * kp : 2 * kp + 2, c0 : c0 + nlen],
                    start=(kp == 0), stop=(kp == 2), perf_mode=DR,
                )

            def kmm(c0, nlen, kp):
                nc.tensor.matmul(
                    ps[:, c0 : c0 + nlen],
                    lhsT=wk_sb[:, 2 * kp : 2 * kp + 2, ms],
                    rhs=xt_t[b][:, 2 * kp : 2 * kp + 2, c0 - 640 : c0 - 640 + nlen],
                    start=(kp == 0), stop=(kp == 2), perf_mode=DR,
                )

            if b == 0 and mt == 0:
                # Q[0:512] (bank0) and K[1024:1408] (bank2) share no PSUM
                # bank, so their accumulation groups may interleave kp-major
                # and fire as each x-chunk DMA lands. The two bank1 groups
                # (Q[512:640], K[640:1024]) must stay chunk-major: a group's
                # start=True re-marks the whole bank pending-zero, so
                # interleaving same-bank groups drops partial sums.
                for kp in range(3):
                    qmm(0, 512, kp)
                    kmm(1024, 384, kp)
                for kp in range(3):
                    qmm(512, 128, kp)
                for kp in range(3):
                    kmm(640, 384, kp)
            else:
                for c0, nlen in ((0, 512), (512, 128)):
                    for kp in range(3):
                        qmm(c0, nlen, kp)
                for c0, nlen in ((640, 384), (1024, 384)):
                    for kp in range(3):
                        kmm(c0, nlen, kp)
            if mt < 5 or b == 1:  # balance: most Q copies on ACT
                nc.scalar.activation(
                    qt_t[b][:, mt, :], ps[:, 0:NQ], AF.Identity,
                    bias=bcq[:, mt : mt + 1], scale=IWS,
                )
            else:
                nc.vector.tensor_scalar(
                    qt_t[b][:, mt, :], ps[:, 0:NQ], IWS, bcq[:, mt : mt + 1],
                    op0=ALU.mult, op1=ALU.add,
                )
            nc.vector.tensor_scalar(
                kt_t[b][:, mt, :], ps[:, 640 : 640 + S], IWS, bck[:, mt : mt + 1],
                op0=ALU.mult, op1=ALU.add,
            )

        def v_tt(b, tt):
            """Candidate-token V tile (fp8)."""
            ts = slice(tt * P, (tt + 1) * P)
            psv = smt("psv")
            for c0, nlen in ((0, 512), (512, 256)):
                for kp in range(3):
                    nc.tensor.matmul(
                        psv[:, c0 : c0 + nlen],
                        lhsT=xt_t[b][:, 2 * kp : 2 * kp + 2, ts],
                        rhs=wv_sb[:, 2 * kp : 2 * kp + 2, c0 : c0 + nlen],
                        start=(kp == 0),
                        stop=(kp == 2),
                        perf_mode=DR,
                    )
            nc.vector.scalar_tensor_tensor(
                out=ve_t[b][:, tt, :, :],
                in0=psv[:, 0:H].rearrange("p (h c) -> p h c", c=DH),
                scalar=IWS,
                in1=bvb.rearrange("p (h c) -> p h c", c=DH),
                op0=ALU.mult,
                op1=ALU.add,
            )

        def v_term(b):
            """Term-token V in bf16 (output passthrough accuracy)."""
            psv = smt("psvt")
            for c0, nlen in ((0, 512), (512, 256)):
                for kt in range(KT):
                    nc.tensor.matmul(
                        psv[:, c0 : c0 + nlen],
                        lhsT=xm_t[b][:, kt, :],
                        rhs=wv16_sb[:, kt, c0 : c0 + nlen],
                        start=(kt == 0),
                        stop=(kt == KT - 1),
                    )
            stg = sg_t[b]
            nc.vector.tensor_tensor(
                out=stg[:, 5, :], in0=psv[:, 0:H], in1=bvb[:], op=ALU.add
            )
            nc.gpsimd.tensor_copy(
                ve_t[b][:, 5, :, :],
                stg[:, 5, :].rearrange("p (h c) -> p h c", c=DH),
            )
            nc.sync.dma_start(
                out=out[b].rearrange("(r p) h -> p r h", p=P)[:, 5, :],
                in_=stg[:, 5, :],
            )

        def scores_h(b, hh):
            mt, hl = hh // 2, hh % 2
            r0 = hl * 64
            KTh = kt_t[b][r0 : r0 + 64, mt, :]
            QTh = qt_t[b][r0 : r0 + 64, mt, :]
            se = se_t[b]
            ph = pjt("ph")
            # se half 0 = block scores (pairs with vext[j] in the PV DR
            # matmul), half 1 = term scores (pairs with vext[5]).
            # zero-fill block region [0:640] (banks 0-1)
            nc.tensor.matmul(
                ph[:, 0:512], lhsT=zpair[:], rhs=xt_t[b][0:1, 0:2, 0:512],
                start=True, stop=False, perf_mode=DR,
            )
            nc.tensor.matmul(
                ph[:, 512:640], lhsT=zpair[:], rhs=xt_t[b][0:1, 0:2, 0:128],
                start=True, stop=False, perf_mode=DR,
            )
            # block-diagonal scores: per q-tile j a [128k x 128q] tile
            for j in range(NJ):
                for half in range(2):
                    c = 2 * j + half
                    nc.tensor.matmul(
                        ph[
                            half * 64 : half * 64 + 64,
                            j * P + half * 64 : j * P + half * 64 + 64,
                        ],
                        lhsT=KTh[:, c * L : (c + 1) * L],
                        rhs=QTh[:, c * L : (c + 1) * L],
                        start=False,
                        stop=(half == 1 and j in (3, 4)),
                    )
            # term scores^T [128 terms, 640 q] in cols [640:1280]
            nc.tensor.matmul(
                ph[:, 640:1024], lhsT=KTh[:, NQ:S], rhs=QTh[:, 0:384],
                start=True, stop=True,
            )
            nc.tensor.matmul(
                ph[:, 1024:1280], lhsT=KTh[:, NQ:S], rhs=QTh[:, 384:640],
                start=True, stop=True,
            )
            # exp (ACT): terms + blocks in ONE instruction (both APs uniform)
            nc.scalar.activation(
                se[:, hh, :, :, :].rearrange("p j two q -> p two j q"),
                ph[:, 0:1280].rearrange("p (two j q) -> p two j q", j=NJ, q=P),
                AF.Exp, scale=0.125,
            )

        def vsums(b):
            """corr'[c] = sum of v over candidate tokens NOT in pair(c)."""
            vef = ve_t[b].rearrange("p k h c -> p k (h c)")
            psc = smt("pscor")
            for c0, nlen in ((0, 512), (512, 256)):
                cs = slice(c0, c0 + nlen)
                for kp in range(2):
                    nc.tensor.matmul(
                        psc[0:64, cs],
                        lhsT=notG6[:, 2 * kp : 2 * kp + 2, :],
                        rhs=vef[:, 2 * kp : 2 * kp + 2, cs],
                        start=(kp == 0),
                        stop=False,
                        perf_mode=DR,
                    )
                nc.tensor.matmul(
                    psc[0:CDD, cs],
                    lhsT=notG6[:, 4, 0:CDD],
                    rhs=vef[:, 4, cs],
                    start=False,
                    stop=True,
                )
            corrE = mp.tile([CDD, ZC + NH], FP8, tag="corr", name=f"corrE_{b}")
            ce_t[b] = corrE
            nc.vector.tensor_copy(corrE[:, 0:ZC], psc[0:CDD, 0:ZC])
            # Z constant: 0.25 * 512 masked-out keys, via indall (0/1 rows)
            nc.gpsimd.memset(corrE[:, ZC : ZC + NH], 128.0)

        def pv_j(b, j, tag, last=False):
            se, vext, corrE, stg = se_t[b], ve_t[b], ce_t[b], sg_t[b]
            psc = pjt("pspv") if tag == "pj" else smt("pspv")
            # corr opens both banks' accumulation groups
            nc.tensor.matmul(
                psc[:, 0:512], lhsT=indall[:, j, :], rhs=corrE[:, 0:512],
                start=True, stop=False,
            )
            nc.tensor.matmul(
                psc[:, 512:768], lhsT=indall[:, j, :], rhs=corrE[:, 512:768],
                start=True, stop=False,
            )
            nc.tensor.matmul(
                psc[:, ZC : ZC + NH], lhsT=indall[:, j, :],
                rhs=corrE[:, ZC : ZC + NH],
                start=False, stop=False,
            )
            for hh in range(NH):
                nc.tensor.matmul(
                    psc[:, hh * DH : (hh + 1) * DH],
                    lhsT=se[:, hh, j, :, :],
                    rhs=vext[:, j : KT : 5 - j, hh, :],
                    start=False,
                    stop=(hh == 7),
                    perf_mode=DR,
                )
                nc.tensor.matmul(
                    psc[:, ZC + hh : ZC + hh + 1],
                    lhsT=se[:, hh, j, :, :],
                    rhs=onecol2[:],
                    start=False,
                    stop=(hh == NH - 1),
                    perf_mode=DR,
                )
            zr = mp.tile([P, NH], FP32, tag="zr", bufs=2, name="zr")
            nc.vector.reciprocal(zr[:], psc[:, ZC : ZC + NH])
            halves = ((0, 6), (6, 12)) if last else ((0, 12),)
            for lo, hi in halves:
                in0 = psc[:, lo * DH : hi * DH].rearrange(
                    "p (h c) -> p h c", c=DH
                )
                in1 = zr[:, lo:hi].rearrange("p (h o) -> p h o", o=1)
                bin0, bin1 = bass.broadcast_tensor_aps(in0, in1)
                nc.vector.scalar_tensor_tensor(
                    out=stg[:, j, lo * DH : hi * DH].rearrange(
                        "p (h c) -> p h c", c=DH
                    ),
                    in0=bin0,
                    scalar=ONECOL,
                    in1=bin1,
                    op0=ALU.mult,
                    op1=ALU.mult,
                )
                nc.sync.dma_start(
                    out=out[b].rearrange("(r p) h -> p r h", p=P)[
                        :, j, lo * DH : hi * DH
                    ],
                    in_=stg[:, j, lo * DH : hi * DH],
                )

        # ---------------- schedule ----------------
        # Scores lag projections by one m-tile so each group's score matmuls
        # overlap the NEXT group's projections instead of waiting on their
        # own Q/K copies. Batch 0 PV hides inside batch 1's loop.
        alloc_bufs(0)
        alloc_bufs(1)

        for mt in range(KT):
            qk_mt(0, mt)
            if mt == 0:
                # bvb[p, o] = bv[o] broadcast (rank-1); after first group so
                # the PE isn't blocked on the bvrow DMA at t=0
                psb = smt("psbv")
                nc.tensor.matmul(psb[:, 0:512], lhsT=onesrow[:], rhs=bvrow[0:1, 0:512], start=True, stop=True)
                nc.tensor.matmul(psb[:, 512:768], lhsT=onesrow[:], rhs=bvrow[0:1, 512:768], start=True, stop=True)
                nc.vector.tensor_copy(bvb[:, 0:512], psb[:, 0:512])
                nc.vector.tensor_copy(bvb[:, 512:768], psb[:, 512:768])
            else:
                scores_h(0, 2 * (mt - 1))
                scores_h(0, 2 * (mt - 1) + 1)
                v_tt(0, mt - 1)
        v_term(0)
        vsums(0)
        scores_h(0, 10)
        scores_h(0, 11)

        for mt in range(KT):
            qk_mt(1, mt)
            if mt >= 2:
                pv_j(0, mt - 2, "pv")
            if mt < 5:
                v_tt(1, mt)
            if mt >= 1:
                scores_h(1, 2 * (mt - 1))
                scores_h(1, 2 * (mt - 1) + 1)
        v_term(1)
        vsums(1)
        scores_h(1, 10)
        scores_h(1, 11)
        pv_j(0, 4, "pv")

        # batch 1 PV, double-buffered via the now-idle 3-bank pj tag
        for j in range(NJ):
            pv_j(1, j, "pj", last=(j == NJ - 1))


_CACHE = {}


def _get_program():
    if "nc" not in _CACHE:
        _CACHE["nc"] = _build_program()
    return _CACHE["nc"]


def _make_in_maps(inputs):
    f8 = ml_dtypes.float8_e4m3
    bf = ml_dtypes.bfloat16
    hs = np.asarray(inputs["hidden_states"], np.float32)
    hst = np.ascontiguousarray(hs.transpose(0, 2, 1))  # [B, H, S]
    wq = np.asarray(inputs["Wq"], np.float32).T * WS
    wk = np.asarray(inputs["Wk"], np.float32).T * WS
    wv = np.asarray(inputs["Wv"], np.float32).T
    in_common = {
        "wq8": np.ascontiguousarray(wq).astype(f8),
        "wk8": np.ascontiguousarray(wk).astype(f8),
        "wv8": np.ascontiguousarray(wv * WS).astype(f8),
        "wv16": np.ascontiguousarray(wv).astype(bf),
        "bq": np.asarray(inputs["bq"], np.float32),
        "bk": np.asarray(inputs["bk"], np.float32),
        "bv16": np.asarray(inputs["bv"], np.float32).astype(bf),
    }
    x8 = hst.astype(f8)
    # xt16 packed [BL, P, KT, T] so the DMA's contiguous runs are 1536B
    xm = (
        hst[:, :, NQ:]
        .reshape(B, KT, P, T)
        .transpose(0, 2, 1, 3)
        .astype(bf)
    )
    return [
        {
            "x8": x8[i * BL : (i + 1) * BL],
            "xt16": np.ascontiguousarray(xm[i * BL : (i + 1) * BL]),
            **in_common,
        }
        for i in range(NCORES)
    ]


def kernel(**inputs) -> np.ndarray:
    in_maps = _make_in_maps(inputs)
    nc = _get_program()
    res = run_bass_kernel_spmd(nc, in_maps, list(range(NCORES)))
    return np.concatenate([res.results[i]["out"] for i in range(NCORES)], axis=0)
